# revision 19
# baseline (speedup 1.0000x reference)
"""Trainium2 Bass kernel for nn_ATVP_router_UNI (moe_routing).

Sharding: output dim D=1536 sharded over 8 cores (192 each). Activations
(x_enc, x_ib) are broadcast in a partition-major transposed layout. The
router's first layer (3072->512) is h-neuron-sharded across cores (64
neurons each; BatchNorm stats are per-neuron so they stay local) with an
AllGather of the post-ReLU bf16 activations. The expert mean over e
commutes with the linear projection, so each core streams its W_proj
slice once (bf16), reduces over e on DVE, and runs 1/7 of the naive
matmul FLOPs in bf16. The softmax denominator and the 1/7 group-mean
scale cancel inside the final l2 normalization and are folded away.
Per-expert biases are batch-loaded, e-reduced on DVE, folded into the
PSUM->SBUF copy (ACT Identity bias).

Scheduling (PE is the measured bottleneck): router group 0 streams
first so the PE starts at ~8us; the uni branch runs after the router
groups; group streams are interleaved with the router finalize so the
PE FIFO never head-of-line blocks on BN; the gated folds run in bf16
against an SBUF-staged gate broadcast (2x DVE mode); the A accumulator
and all tail intermediates are bf16. Cross-core coupling: AllGather of
h1 (hidden under streaming), AllReduce of the uni sum-of-squares
(hidden), and the 4KB out-norm AllReduce on the tail (the t*z fold
runs during its flight).

kernel(**inputs) takes the full unsharded inputs and returns the full
[1024, 1536] f32 output. Host-side prep does layout/dtype staging only --
all arithmetic runs on device.
"""

import numpy as np
import ml_dtypes

import concourse.bass as bass
import concourse.tile as tile
import concourse.mybir as mybir
from concourse import bacc
from concourse.bass_utils import run_bass_kernel_spmd

f32 = mybir.dt.float32
f32r = mybir.dt.float32r
bf16 = mybir.dt.bfloat16
AX = mybir.AxisListType
ALU = mybir.AluOpType
ACTF = mybir.ActivationFunctionType

NCORES = 8
B, N, G, K, D = 1024, 10, 7, 1024, 1536
DS = D // NCORES          # 192 output dims per core
KT = K // 128             # 8 k-tiles
KH = KT // 2              # 4 k-tiles per W half-transfer
BSL = 512                 # free-dim slice for matmuls (PSUM bank limit)
NBS = B // BSL            # 2
HS = 512 // NCORES        # 64 router h1 neurons per core
ROUTER_GS = (0, 4, 8)     # TEXT_PRED groups, streamed first
CHUNKS = ((0, 128), (128, 64))  # (d-offset, size) chunks of DS=192
EPS_BN = 1e-5
EPS_NORM = 1e-12

LAST_RESULTS = None
_NC_CACHE = {}


def _emit(nc, tc):
    # ---- DRAM I/O ----
    xT_d = nc.dram_tensor("xT", [N, 128, KT, B], bf16, kind="ExternalInput").ap()
    xibT_d = nc.dram_tensor("xibT", [128, KT, B], bf16, kind="ExternalInput").ap()
    w_d = nc.dram_tensor("w", [N, 128, KT, G, DS], bf16, kind="ExternalInput").ap()
    wib_d = nc.dram_tensor("wib", [128, KT, G, DS], bf16, kind="ExternalInput").ap()
    xuT_d = nc.dram_tensor("xuT", [DS, B], bf16, kind="ExternalInput").ap()
    bp0_d = nc.dram_tensor("bp0", [128, N + 1, G], bf16, kind="ExternalInput").ap()
    bp1_d = nc.dram_tensor("bp1", [64, N + 1, G], bf16, kind="ExternalInput").ap()
    rw1_d = nc.dram_tensor("rw1", [128, 3, KT, HS], bf16, kind="ExternalInput").ap()
    rw2_d = nc.dram_tensor("rw2", [HS, 100], bf16, kind="ExternalInput").ap()
    rw3_d = nc.dram_tensor("rw3", [100, 11], bf16, kind="ExternalInput").ap()
    rb3_d = nc.dram_tensor("rb3", [11, 1], f32, kind="ExternalInput").ap()
    sel_d = nc.dram_tensor("sel", [11, 11, 128], f32r, kind="ExternalInput").ap()
    ones_d = nc.dram_tensor("onesd", [1, 128], f32r, kind="ExternalInput").ap()
    onesb_d = nc.dram_tensor("onesb16", [128, 2], bf16, kind="ExternalInput").ap()
    outT_d = nc.dram_tensor("outT", [DS, B], bf16, kind="ExternalOutput").ap()

    pools = {}

    def pool(name, bufs, space="SBUF"):
        cm = tc.tile_pool(name=name, bufs=bufs, space=space)
        pools[name] = cm
        return cm.__enter__()

    cp = pool("const", 1)       # persistent constants / accumulators
    xtp = pool("xt", 3)         # [128, KT, B] bf16 whole-group x
    wtp = pool("wt", 2)         # [128, KH, G, DS] bf16 half-group W
    wsp = pool("ws", 2)         # [128, KT, DS] bf16 e-reduced weights
    wrp = pool("wred", 3)       # [128, KH, DS] bf16 e-reduce tree temps
    gs0 = pool("gs0", 8)        # [128, B] bf16 staged raw (chunk 0)
    gs1 = pool("gs1", 8)        # [64, B] bf16 staged raw (chunk 1)
    bcp = pool("bc", 2)         # [128, B] bf16 staged gate broadcast
    b1k = pool("big1k", 2)      # [128, B] f32 scratch (bn dumps)
    ctb = pool("ctb", 4)        # [128, B] bf16 scratch (squares/folds)
    ctp = pool("ctmp", 3)       # [128, BSL] f32 scratch
    stp = pool("stat", 10)      # [128, 1] BN stats smalls
    psp = pool("ps", 8, space="PSUM")
    drp = pool("dram", 1, space="DRAM")

    def ps_tile(p, n=BSL):
        return psp.tile([p, n], f32, tag="ps", name="ps")

    # ---- constants (gpsimd SWDGE queue; sync queue kept clear for x/W) ----
    ones1 = cp.tile([1, 128], f32r, tag="ones1", name="ones1")
    nc.gpsimd.dma_start(ones1[:], ones_d[:])
    onesb = cp.tile([128, 1], bf16, tag="onesb", name="onesb")
    nc.gpsimd.dma_start(onesb[:], onesb_d[:, 0:1])
    twosb = cp.tile([128, 1], bf16, tag="twosb", name="twosb")
    nc.gpsimd.dma_start(twosb[:], onesb_d[:, 1:2])
    sel = cp.tile([11, 11, 128], f32r, tag="sel", name="sel")
    nc.gpsimd.dma_start(sel[:], sel_d[:])
    rb3 = cp.tile([11, 1], f32, tag="rb3", name="rb3")
    nc.gpsimd.dma_start(rb3[:], rb3_d[:])
    rw2 = cp.tile([HS, 100], bf16, tag="rw2", name="rw2")
    nc.gpsimd.dma_start(rw2[:], rw2_d[:])
    rw3 = cp.tile([100, 11], bf16, tag="rw3", name="rw3")
    nc.gpsimd.dma_start(rw3[:], rw3_d[:])
    bp0 = cp.tile([128, N + 1, G], bf16, tag="bp0", name="bp0")
    nc.gpsimd.dma_start(bp0[:], bp0_d[:])
    bp1 = cp.tile([64, N + 1, G], bf16, tag="bp1", name="bp1")
    nc.gpsimd.dma_start(bp1[:], bp1_d[:])

    # rw1 slice first on the sync queue: h1 matmuls only need rt + xT[0]
    rt = cp.tile([128, 3, KT, HS], bf16, tag="rt", name="rt")
    nc.sync.dma_start(rt[:], rw1_d[:])

    # bias e-reduce: [p, 11, G] -> [p, 11] f32 columns
    bsum0 = cp.tile([128, N + 1], f32, tag="bsum0", name="bsum0")
    nc.vector.reduce_sum(out=bsum0[:], in_=bp0[:], axis=AX.X)
    bsum1 = cp.tile([64, N + 1], f32, tag="bsum1", name="bsum1")
    nc.vector.reduce_sum(out=bsum1[:], in_=bp1[:], axis=AX.X)
    bsums = (bsum0, bsum1)
    # uni bias column pre-scaled by 0.1/7
    bib_sc = []
    for ci, (m0, msz) in enumerate(CHUNKS):
        c2 = stp.tile([128, 1], f32, tag="stat", name="stat")
        nc.vector.tensor_scalar_mul(c2[0:msz, :], bsums[ci][0:msz, N:N + 1],
                                    0.1 / 7.0)
        bib_sc.append(c2)

    # ---- persistent buffers ----
    h1loc = cp.tile([HS, B], f32, tag="h1loc", name="h1loc")
    h2g = cp.tile([100, B], bf16, tag="h2g", name="h2g")
    A = [cp.tile([msz, B], bf16, tag=f"A{ci}", name=f"A{ci}")
         for ci, (m0, msz) in enumerate(CHUNKS)]
    z = [cp.tile([msz, B], bf16, tag=f"z{ci}", name=f"z{ci}")
         for ci, (m0, msz) in enumerate(CHUNKS)]
    outsb = [cp.tile([msz, B], bf16, tag=f"out{ci}", name=f"out{ci}")
             for ci, (m0, msz) in enumerate(CHUNKS)]
    h2b = cp.tile([100, B], bf16, tag="h2b", name="h2b")
    ex10 = cp.tile([11, B], f32r, tag="ex10", name="ex10")
    arv = cp.tile([1, 3 * B], f32, tag="arv", name="arv")
    tv = cp.tile([1, B], f32r, tag="tv", name="tv")
    uv = cp.tile([1, B], f32r, tag="uv", name="uv")
    xu = []
    for ci, (m0, msz) in enumerate(CHUNKS):
        xu.append(cp.tile([msz, B], bf16, tag=f"xu{ci}", name=f"xu{ci}"))

    cc1_in = drp.tile([1, B], f32, tag="cc1_in", name="cc1_in")
    # AR1 result laid out [128, 8] so the 1/||U|| reciprocal runs 128-wide
    # (a [1, B] row reciprocal is a 6.5us single-partition stall that Tile
    # schedules mid-pipeline on the DVE queue, starving the PE)
    cc1_out = drp.tile([128, B // 128], f32, tag="cc1_out",
                       addr_space="Shared", name="cc1_out")
    us_dram = drp.tile([1, B], f32r, tag="us_dram", name="us_dram")
    cc2_in = drp.tile([1, B], f32, tag="cc2_in", name="cc2_in")
    # AR2 result laid out [128, 8] so the tail rsqrt runs 128-wide
    cc2_out = drp.tile([128, B // 128], f32, tag="cc2_out",
                       addr_space="Shared", name="cc2_out")
    cc3_in = drp.tile([100, B], bf16, tag="cc3_in", name="cc3_in")
    cc3_out = drp.tile([100, B], bf16, tag="cc3_out",
                       addr_space="Shared", name="cc3_out")
    u_dram = drp.tile([1, B], f32r, tag="u_dram", name="u_dram")

    def stream_group(xsrc, wsrc):
        """DMA one group's x (one shot) + W (two halves), reduce W over e.

        The e-reduction is a pairwise add tree with expert slices contiguous
        along DS (step-1 bf16 operands hit the DVE 2x mode).
        """
        xt = xtp.tile([128, KT, B], bf16, tag="xt", name="xt")
        nc.sync.dma_start(xt[:], xsrc)
        ws = wsp.tile([128, KT, DS], bf16, tag="ws", name="ws")
        for h in range(2):
            wt = wtp.tile([128, KH, G, DS], bf16, tag="wt", name="wt")
            nc.sync.dma_start(wt[:], wsrc[:, h * KH:(h + 1) * KH, :, :])
            t01 = wrp.tile([128, KH, DS], bf16, tag="wred", name="wred")
            nc.vector.tensor_tensor(out=t01[:], in0=wt[:, :, 0, :],
                                    in1=wt[:, :, 1, :], op=ALU.add)
            t23 = wrp.tile([128, KH, DS], bf16, tag="wred", name="wred")
            nc.vector.tensor_tensor(out=t23[:], in0=wt[:, :, 2, :],
                                    in1=wt[:, :, 3, :], op=ALU.add)
            t45 = wrp.tile([128, KH, DS], bf16, tag="wred", name="wred")
            nc.vector.tensor_tensor(out=t45[:], in0=wt[:, :, 4, :],
                                    in1=wt[:, :, 5, :], op=ALU.add)
            nc.vector.tensor_tensor(out=t01[:], in0=t01[:], in1=t23[:],
                                    op=ALU.add)
            nc.vector.tensor_tensor(out=t45[:], in0=t45[:], in1=wt[:, :, 6, :],
                                    op=ALU.add)
            nc.vector.tensor_tensor(out=ws[:, h * KH:(h + 1) * KH, :],
                                    in0=t01[:], in1=t45[:], op=ALU.add)
        return xt, ws

    def group_matmuls(xt, ws, g):
        """raw = x @ Wsum (+ e-summed bias on the ACT copy); stage bf16."""
        raws = []
        for ci, (m0, msz) in enumerate(CHUNKS):
            gp = gs0 if ci == 0 else gs1
            raw = gp.tile([msz, B], bf16, tag=f"gs{ci}", name=f"gs{ci}")
            for bs in range(NBS):
                sl = slice(bs * BSL, (bs + 1) * BSL)
                ps = ps_tile(msz)
                for kt in range(KT):
                    nc.tensor.matmul(
                        ps[:],
                        lhsT=ws[:, kt, m0:m0 + msz],
                        rhs=xt[:, kt, sl],
                        start=(kt == 0), stop=(kt == KT - 1))
                nc.scalar.activation(raw[:, sl], ps[:], ACTF.Identity,
                                     bias=bsums[ci][0:msz, g:g + 1])
            raws.append(raw)
        return raws

    def h1_mms(ri, xt):
        """Local h1 partial for router group ri: [HS, B] over this core's
        64 neurons (contraction over this group's 1024 k)."""
        for bs in range(NBS):
            sl = slice(bs * BSL, (bs + 1) * BSL)
            ps = ps_tile(HS)
            for kt in range(KT):
                nc.tensor.matmul(
                    ps[:],
                    lhsT=rt[:, ri, kt, :],
                    rhs=xt[:, kt, sl],
                    start=(kt == 0), stop=(kt == KT - 1))
            if ri == 0:
                nc.scalar.copy(h1loc[:, sl], ps[:])
            else:
                nc.vector.tensor_tensor(out=h1loc[:, sl], in0=ps[:],
                                        in1=h1loc[:, sl], op=ALU.add)

    fold_state = {"first": True}

    def fold_group(raws, i):
        """A += e_i * raw. Gate row i is PE-broadcast to 128 partitions,
        ACT-staged to SBUF bf16 so the DVE folds run in 2x mode."""
        bcs = bcp.tile([128, B], bf16, tag="bc", name="bc")
        for bs in range(NBS):
            sl = slice(bs * BSL, (bs + 1) * BSL)
            bc = ps_tile(128)
            nc.tensor.matmul(bc[:], lhsT=sel[:, i, :],
                             rhs=ex10[:, sl], start=True, stop=True)
            nc.scalar.copy(bcs[:, sl], bc[:])
        first = fold_state["first"]
        fold_state["first"] = False
        # chunk 0 on DVE, chunk 1 on the otherwise-idle GpSimd
        for eng, (ci, (m0, msz)) in zip((nc.vector, nc.gpsimd),
                                        enumerate(CHUNKS)):
            if first:
                eng.tensor_tensor(out=A[ci][:], in0=raws[ci][:],
                                  in1=bcs[0:msz, :], op=ALU.mult)
            else:
                tmp = ctb.tile([128, B], bf16, tag="ctb", name="ctb")
                eng.tensor_tensor(out=tmp[0:msz, :], in0=raws[ci][:],
                                  in1=bcs[0:msz, :], op=ALU.mult)
                eng.tensor_tensor(out=A[ci][:], in0=A[ci][:],
                                  in1=tmp[0:msz, :], op=ALU.add)

    def bn_act(tiles, out_tiles, nparts, func):
        """BatchNorm (training stats over free axis) + activation."""
        for t, to in zip(tiles, out_tiles):
            dump = b1k.tile([128, B], f32, tag="big1k", name="big1k")
            mnr = stp.tile([128, 1], f32, tag="stat", name="stat")
            nc.scalar.activation(dump[0:nparts, :], t[:], ACTF.Copy,
                                 accum_out=mnr[0:nparts, :])
            mn = stp.tile([128, 1], f32, tag="stat", name="stat")
            nc.scalar.mul(mn[0:nparts, :], mnr[0:nparts, :], 1.0 / B)
            sq = b1k.tile([128, B], f32, tag="big1k", name="big1k")
            ex2r = stp.tile([128, 1], f32, tag="stat", name="stat")
            nc.scalar.activation(sq[0:nparts, :], t[:], ACTF.Square,
                                 accum_out=ex2r[0:nparts, :])
            ex2 = stp.tile([128, 1], f32, tag="stat", name="stat")
            nc.scalar.mul(ex2[0:nparts, :], ex2r[0:nparts, :], 1.0 / B)
            var = stp.tile([128, 1], f32, tag="stat", name="stat")
            nc.vector.tensor_tensor(out=var[0:nparts, :], in0=mn[0:nparts, :],
                                    in1=mn[0:nparts, :], op=ALU.mult)
            nc.vector.tensor_tensor(out=var[0:nparts, :], in0=ex2[0:nparts, :],
                                    in1=var[0:nparts, :], op=ALU.subtract)
            nc.vector.tensor_scalar_add(var[0:nparts, :], var[0:nparts, :],
                                        EPS_BN)
            sd = stp.tile([128, 1], f32, tag="stat", name="stat")
            nc.scalar.sqrt(sd[0:nparts, :], var[0:nparts, :])
            rs = stp.tile([128, 1], f32, tag="stat", name="stat")
            nc.vector.reciprocal(rs[0:nparts, :], sd[0:nparts, :])
            nb = stp.tile([128, 1], f32, tag="stat", name="stat")
            nc.vector.tensor_tensor(out=nb[0:nparts, :], in0=mn[0:nparts, :],
                                    in1=rs[0:nparts, :], op=ALU.mult)
            nc.vector.tensor_scalar_mul(nb[0:nparts, :], nb[0:nparts, :], -1.0)
            nc.scalar.activation(to[:], t[:], func,
                                 bias=nb[0:nparts, :], scale=rs[0:nparts, :])

    # ================= router groups first (PE starts on h1 at ~8us) ======
    router_raws = {}
    for ri, g in enumerate(ROUTER_GS):
        xt, ws = stream_group(xT_d[g], w_d[g])
        h1_mms(ri, xt)
        router_raws[g] = group_matmuls(xt, ws, g)

    # bn+relu on local h1 slice, then this core's h2 partial through its
    # 64-row slice of rw2; the cross-core coupling is a small [100, B]
    # bf16 AllReduce instead of a 1MB h1 AllGather (bcp pool is not
    # contended until the folds start, long after this is consumed)
    h1b_loc = bcp.tile([128, B], bf16, tag="bc", name="bc")
    bn_act([h1loc], [h1b_loc[0:HS, :]], HS, ACTF.Relu)
    h2p = ctb.tile([128, B], bf16, tag="ctb", name="ctb")
    for bs in range(NBS):
        sl = slice(bs * BSL, (bs + 1) * BSL)
        ps = ps_tile(100)
        nc.tensor.matmul(ps[:], lhsT=rw2[:], rhs=h1b_loc[0:HS, sl],
                         start=True, stop=True)
        nc.scalar.copy(h2p[0:100, sl], ps[:])
    nc.sync.dma_start(cc3_in[:], h2p[0:100, :])
    nc.gpsimd.collective_compute(
        "AllReduce", ALU.add,
        ins=[cc3_in.opt()], outs=[cc3_out.opt()],
        replica_groups=[list(range(NCORES))])

    # ================= uni branch -> z, ssz partial, AR1 =================
    for ci, (m0, msz) in enumerate(CHUNKS):
        nc.sync.dma_start(xu[ci][:], xuT_d[m0:m0 + msz, :])
    xib_t, wib_s = stream_group(xibT_d, wib_d)
    for ci, (m0, msz) in enumerate(CHUNKS):
        for bs in range(NBS):
            sl = slice(bs * BSL, (bs + 1) * BSL)
            ps = ps_tile(msz)
            for kt in range(KT):
                nc.tensor.matmul(ps[:], lhsT=wib_s[:, kt, m0:m0 + msz],
                                 rhs=xib_t[:, kt, sl],
                                 start=(kt == 0), stop=(kt == KT - 1))
            # z = (raw + bias)*0.1/7 + 0.9*xu  (fused add on DVE)
            nc.scalar.activation(z[ci][:, sl], ps[:], ACTF.Identity,
                                 scale=0.1 / 7.0, bias=bib_sc[ci][0:msz, :])
            nc.vector.scalar_tensor_tensor(
                out=z[ci][:, sl], in0=xu[ci][:, sl], scalar=0.9,
                in1=z[ci][:, sl], op0=ALU.mult, op1=ALU.add)
    # ssz partial: sum_d z^2 -> arv slot 2 -> cc1_in
    for bs in range(NBS):
        sl = slice(bs * BSL, (bs + 1) * BSL)
        ps = ps_tile(1)
        for ci, (m0, msz) in enumerate(CHUNKS):
            sq = ctb.tile([128, B], bf16, tag="ctb", name="ctb")
            nc.scalar.square(sq[0:msz, 0:BSL], z[ci][:, sl])
            nc.tensor.matmul(ps[:], lhsT=onesb[0:msz, :],
                             rhs=sq[0:msz, 0:BSL],
                             start=(ci == 0), stop=(ci == len(CHUNKS) - 1))
        nc.scalar.copy(arv[:, B + bs * BSL:B + (bs + 1) * BSL], ps[:])
    nc.sync.dma_start(cc1_in[:], arv[:, B:2 * B])
    nc.gpsimd.collective_compute(
        "AllReduce", ALU.add,
        ins=[cc1_in.opt()], outs=[cc1_out.opt()],
        replica_groups=[list(range(NCORES))])

    # ===== stream groups 1,2,3,5 (PE cover for bn/AG/finalize latency) ====
    pre_raws = {}
    for g in (1, 2, 3):
        xt, ws = stream_group(xT_d[g], w_d[g])
        pre_raws[g] = group_matmuls(xt, ws, g)

    # ================= router finalize =================
    # h2 AllReduce result fetched on the gpsimd queue: its completion wait
    # must not head-of-line-block the sync (x/W streams) or scalar (ACT
    # staging) queues, and gpsimd has nothing urgent queued behind it.
    nc.gpsimd.dma_start(h2g[:], cc3_out[:])
    g5_xt, g5_ws = stream_group(xT_d[5], w_d[5])
    pre_raws[5] = group_matmuls(g5_xt, g5_ws, 5)
    bn_act([h2g], [h2b], 100, ACTF.Tanh)
    for bs in range(NBS):
        sl = slice(bs * BSL, (bs + 1) * BSL)
        ps = ps_tile(11)
        nc.tensor.matmul(ps[:], lhsT=rw3[:],
                         rhs=h2b[:, sl], start=True, stop=True)
        sg = ctp.tile([128, BSL], f32, tag="ctmp", name="ctmp")
        nc.scalar.activation(sg[0:11, :], ps[:], ACTF.Sigmoid, bias=rb3[:],
                             scale=1.0)
        nc.scalar.activation(ex10[:, sl], sg[0:11, :], ACTF.Exp, scale=10.0)

    # e10 row (no AR1 dependency): arv slot 3 <- 7*e10
    for bs in range(NBS):
        sl = slice(bs * BSL, (bs + 1) * BSL)
        ps = ps_tile(1)
        nc.tensor.matmul(ps[:], lhsT=sel[:, 10, 0:1],
                         rhs=ex10[:, sl], start=True, stop=True)
        nc.scalar.copy(arv[:, 2 * B + bs * BSL:2 * B + (bs + 1) * BSL], ps[:])
    e10v = arv[:, 2 * B:3 * B]
    nc.vector.tensor_scalar_mul(e10v, e10v, 7.0)

    # ================= folds: staged groups, then stream the rest =========
    for g in (0, 4, 8, 1, 2, 3):
        fold_group(router_raws.get(g) or pre_raws[g], g)
    fold_group(pre_raws[5], 5)

    for g in (6, 7, 9):
        xt, ws = stream_group(xT_d[g], w_d[g])
        raws = group_matmuls(xt, ws, g)
        fold_group(raws, g)

    # AR1 result -> 1/||U|| on [128, 8] (cheap 128-wide recip), bounced
    # through DRAM back into [1, B] row form for the gate-row arithmetic.
    # All on the gpsimd queue, which has nothing urgent behind this.
    s8 = cp.tile([128, B // 128], f32, tag="s8", name="s8")
    nc.gpsimd.dma_start(s8[:], cc1_out[:])
    nc.scalar.sqrt(s8[:], s8[:])
    nc.vector.tensor_scalar_max(s8[:], s8[:], EPS_NORM)
    s8r = cp.tile([128, B // 128], f32r, tag="s8r", name="s8r")
    nc.vector.reciprocal(s8r[:], s8[:])
    nc.gpsimd.dma_start(us_dram[:], s8r[:])
    nc.gpsimd.dma_start(uv[:], us_dram[:])
    nc.vector.tensor_tensor(out=tv[:], in0=e10v, in1=uv[:], op=ALU.mult)
    # e10sq/8: each core contributes 1/8 so the AR2 sum restores it
    nc.vector.tensor_tensor(out=e10v, in0=e10v, in1=e10v, op=ALU.mult)
    nc.vector.tensor_scalar_mul(e10v, e10v, 0.125)

    # ========= tail: q_loc = |A|^2 + t*(2 A.z) + (7e10)^2/8 -> AR2 ========
    for bs in range(NBS):
        sl = slice(bs * BSL, (bs + 1) * BSL)
        psa = ps_tile(1)
        psc = ps_tile(1)
        for ci, (m0, msz) in enumerate(CHUNKS):
            sqa = ctb.tile([128, B], bf16, tag="ctb", name="ctb")
            nc.scalar.square(sqa[0:msz, 0:BSL], A[ci][:, sl])
            nc.tensor.matmul(psa[:], lhsT=onesb[0:msz, :],
                             rhs=sqa[0:msz, 0:BSL],
                             start=(ci == 0), stop=(ci == len(CHUNKS) - 1))
            cza = ctb.tile([128, B], bf16, tag="ctb", name="ctb")
            nc.vector.tensor_tensor(out=cza[0:msz, 0:BSL], in0=A[ci][:, sl],
                                    in1=z[ci][:, sl], op=ALU.mult)
            nc.tensor.matmul(psc[:], lhsT=twosb[0:msz, :],
                             rhs=cza[0:msz, 0:BSL],
                             start=(ci == 0), stop=(ci == len(CHUNKS) - 1))
        qt = ctp.tile([128, BSL], f32, tag="ctmp", name="ctmp")
        nc.vector.tensor_tensor(out=qt[0:1, :], in0=psc[:], in1=tv[:, sl],
                                op=ALU.mult)
        nc.vector.tensor_tensor(out=qt[0:1, :], in0=qt[0:1, :], in1=psa[:],
                                op=ALU.add)
        nc.vector.tensor_tensor(out=arv[:, bs * BSL:(bs + 1) * BSL],
                                in0=qt[0:1, :],
                                in1=arv[:, 2 * B + bs * BSL:2 * B + (bs + 1) * BSL],
                                op=ALU.add)
    nc.sync.dma_start(cc2_in[:], arv[:, 0:B])
    nc.gpsimd.collective_compute(
        "AllReduce", ALU.add,
        ins=[cc2_in.opt()], outs=[cc2_out.opt()],
        replica_groups=[list(range(NCORES))])

    # P = A + t*z, computed while AR2 is in flight
    tvb = bcp.tile([128, B], bf16, tag="bc", name="bc")
    for bs in range(NBS):
        sl = slice(bs * BSL, (bs + 1) * BSL)
        btv = ps_tile(128)
        nc.tensor.matmul(btv[:], lhsT=ones1[:],
                         rhs=tv[:, sl], start=True, stop=True)
        nc.scalar.copy(tvb[:, sl], btv[:])
    for ci, (m0, msz) in enumerate(CHUNKS):
        tmp = ctb.tile([128, B], bf16, tag="ctb", name="ctb")
        nc.vector.tensor_tensor(out=tmp[0:msz, :], in0=z[ci][:],
                                in1=tvb[0:msz, :], op=ALU.mult)
        nc.vector.tensor_tensor(out=A[ci][:], in0=A[ci][:],
                                in1=tmp[0:msz, :], op=ALU.add)

    # sqrt ACT table preload during AR2 flight: the tail sqrt otherwise
    # pays a ~1.5us ACT_TABLE_LOAD on the critical path
    dwarm = stp.tile([128, 1], f32, tag="stat", name="stat")
    nc.scalar.sqrt(dwarm[0:1, :], bib_sc[0][0:1, :])

    # q -> u = 1/max(sqrt(q), eps) on [128, 8] (a [1, B] row reciprocal is
    # a 7.8us single-partition iterative divide; 128-wide it is ~0.1us),
    # then a DRAM bounce turns u back into a [1, B] row for the broadcast
    u8 = cp.tile([128, B // 128], f32, tag="u8", name="u8")
    nc.sync.dma_start(u8[:], cc2_out[:])
    nc.scalar.sqrt(u8[:], u8[:])
    nc.vector.tensor_scalar_max(u8[:], u8[:], EPS_NORM)
    u8r = cp.tile([128, B // 128], f32r, tag="u8r", name="u8r")
    nc.vector.reciprocal(u8r[:], u8[:])
    nc.sync.dma_start(u_dram[:], u8r[:])
    ubrow = cp.tile([1, B], f32r, tag="ubrow", name="ubrow")
    nc.sync.dma_start(ubrow[:], u_dram[:])
    ub = bcp.tile([128, B], bf16, tag="bc", name="bc")
    for bs in range(NBS):
        sl = slice(bs * BSL, (bs + 1) * BSL)
        bu = ps_tile(128)
        nc.tensor.matmul(bu[:], lhsT=ones1[:],
                         rhs=ubrow[:, sl], start=True, stop=True)
        nc.scalar.copy(ub[:, sl], bu[:])
    for ci, (m0, msz) in enumerate(CHUNKS):
        nc.vector.tensor_tensor(out=outsb[ci][:], in0=A[ci][:],
                                in1=ub[0:msz, :], op=ALU.mult)
        nc.sync.dma_start(outT_d[m0:m0 + msz, :], outsb[ci][:])

    for p in reversed(list(pools.values())):
        p.__exit__(None, None, None)


def _build_nc():
    nc = bacc.Bacc("TRN2", target_bir_lowering=False, debug=False,
                   num_devices=NCORES)
    with tile.TileContext(nc) as tc:
        with nc.allow_low_precision(reason="bf16 streams / f32r reductions are intentional"):
            _emit(nc, tc)
    nc.compile()
    return nc


def _as_bf16(a):
    return np.ascontiguousarray(a.astype(ml_dtypes.bfloat16))


def _host_prep(inputs):
    x_enc = np.asarray(inputs["x_enc"], dtype=np.float32)
    x_ib = np.asarray(inputs["x_ib"], dtype=np.float32)
    x_uni = np.asarray(inputs["x_uni"], dtype=np.float32)
    W_proj = np.asarray(inputs["W_proj"], dtype=np.float32)
    b_proj = np.asarray(inputs["b_proj"], dtype=np.float32)
    W_ib = np.asarray(inputs["W_ib"], dtype=np.float32)
    b_ib = np.asarray(inputs["b_ib"], dtype=np.float32)

    # x_enc [N,B,K] -> [N, 128, KT, B] partition-major bf16
    xT = _as_bf16(x_enc.transpose(0, 2, 1).reshape(N, KT, 128, B).transpose(0, 2, 1, 3))
    # x_ib [B,K] -> [128, KT, B]
    xibT = _as_bf16(x_ib.T.reshape(KT, 128, B).transpose(1, 0, 2))
    sel = np.zeros((11, 11, 128), dtype=np.float32)
    for q in range(11):
        sel[q, q, :] = 1.0
    rb3 = np.ascontiguousarray(np.asarray(inputs["r_b3"], np.float32).reshape(11, 1))
    # r_w1 [3072, 512]: per-core h-slice -> [128, 3, KT, 64]
    rw1_full = np.asarray(inputs["r_w1"], np.float32)
    rw2_full = np.asarray(inputs["r_w2"], np.float32)
    rw3 = _as_bf16(np.asarray(inputs["r_w3"], np.float32))
    ones_host = np.ones((1, 128), dtype=np.float32)
    onesb16 = np.ones((128, 2), dtype=np.float32)
    onesb16[:, 1] = 2.0
    onesb16 = _as_bf16(onesb16)

    in_maps = []
    for c in range(NCORES):
        ds = slice(c * DS, (c + 1) * DS)
        # W_proj [N,G,K,D] ds-slice -> [N, 128, KT, G, DS]
        wc = _as_bf16(W_proj[:, :, :, ds].reshape(N, G, KT, 128, DS)
                      .transpose(0, 3, 2, 1, 4))
        wibc = _as_bf16(W_ib[:, :, ds].reshape(G, KT, 128, DS)
                        .transpose(2, 1, 0, 3))
        # biases: [DS, 11, G] with group 10 = b_ib, split into chunks
        bp_full = np.zeros((DS, N + 1, G), dtype=np.float32)
        bp_full[:, :N, :] = b_proj[:, :, ds].transpose(2, 0, 1)
        bp_full[:, N, :] = b_ib[:, ds].T
        bp_full = _as_bf16(bp_full)
        rw1c = _as_bf16(rw1_full[:, c * HS:(c + 1) * HS]
                        .reshape(3, KT, 128, HS).transpose(2, 0, 1, 3))
        rw2c = _as_bf16(rw2_full[c * HS:(c + 1) * HS, :])
        in_maps.append({
            "xT": xT,
            "xibT": xibT,
            "w": wc,
            "wib": wibc,
            "xuT": _as_bf16(x_uni[:, ds].T),
            "bp0": bp_full[0:128],
            "bp1": bp_full[128:192],
            "rw1": rw1c,
            "rw2": rw2c,
            "rw3": rw3,
            "rb3": rb3,
            "sel": sel,
            "onesd": ones_host,
            "onesb16": onesb16,
        })
    return in_maps


def kernel(**inputs):
    global LAST_RESULTS
    if "nc" not in _NC_CACHE:
        _NC_CACHE["nc"] = _build_nc()
    nc = _NC_CACHE["nc"]
    in_maps = _host_prep(inputs)
    res = run_bass_kernel_spmd(nc, in_maps, list(range(NCORES)))
    LAST_RESULTS = res
    full = np.concatenate(
        [res.results[c]["outT"].astype(np.float32) for c in range(NCORES)],
        axis=0)
    return np.ascontiguousarray(full.T)


# revision 20
# speedup vs baseline: 1.0386x; 1.0386x over previous
"""Trainium2 Bass kernel for nn_ATVP_router_UNI (moe_routing).

Sharding: output dim D=1536 sharded over 8 cores (192 each). Activations
(x_enc, x_ib) are broadcast in a partition-major transposed layout. The
router's first layer (3072->512) is h-neuron-sharded across cores (64
neurons each; BatchNorm stats are per-neuron so they stay local) with an
AllGather of the post-ReLU bf16 activations. The expert mean over e
commutes with the linear projection, so each core streams its W_proj
slice once (bf16), reduces over e on DVE, and runs 1/7 of the naive
matmul FLOPs in bf16. The softmax denominator and the 1/7 group-mean
scale cancel inside the final l2 normalization and are folded away.
Per-expert biases are batch-loaded, e-reduced on DVE, folded into the
PSUM->SBUF copy (ACT Identity bias).

Scheduling (PE is the measured bottleneck): router group 0 streams
first so the PE starts at ~8us; the uni branch runs after the router
groups; group streams are interleaved with the router finalize so the
PE FIFO never head-of-line blocks on BN; the gated folds run in bf16
against an SBUF-staged gate broadcast (2x DVE mode); the A accumulator
and all tail intermediates are bf16. Cross-core coupling: AllGather of
h1 (hidden under streaming), AllReduce of the uni sum-of-squares
(hidden), and the 4KB out-norm AllReduce on the tail (the t*z fold
runs during its flight).

kernel(**inputs) takes the full unsharded inputs and returns the full
[1024, 1536] f32 output. Host-side prep does layout/dtype staging only --
all arithmetic runs on device.
"""

import numpy as np
import ml_dtypes

import concourse.bass as bass
import concourse.tile as tile
import concourse.mybir as mybir
from concourse import bacc
from concourse.bass_utils import run_bass_kernel_spmd

f32 = mybir.dt.float32
f32r = mybir.dt.float32r
bf16 = mybir.dt.bfloat16
AX = mybir.AxisListType
ALU = mybir.AluOpType
ACTF = mybir.ActivationFunctionType

NCORES = 8
B, N, G, K, D = 1024, 10, 7, 1024, 1536
DS = D // NCORES          # 192 output dims per core
KT = K // 128             # 8 k-tiles
KH = KT // 2              # 4 k-tiles per W half-transfer
BSL = 512                 # free-dim slice for matmuls (PSUM bank limit)
NBS = B // BSL            # 2
HS = 512 // NCORES        # 64 router h1 neurons per core
ROUTER_GS = (0, 4, 8)     # TEXT_PRED groups, streamed first
CHUNKS = ((0, 128), (128, 64))  # (d-offset, size) chunks of DS=192
EPS_BN = 1e-5
EPS_NORM = 1e-12

LAST_RESULTS = None
_NC_CACHE = {}


def _emit(nc, tc):
    # ---- DRAM I/O ----
    xT_d = nc.dram_tensor("xT", [N, 128, KT, B], bf16, kind="ExternalInput").ap()
    xibT_d = nc.dram_tensor("xibT", [128, KT, B], bf16, kind="ExternalInput").ap()
    w_d = nc.dram_tensor("w", [N, 128, KT, G, DS], bf16, kind="ExternalInput").ap()
    wib_d = nc.dram_tensor("wib", [128, KT, G, DS], bf16, kind="ExternalInput").ap()
    xuT_d = nc.dram_tensor("xuT", [DS, B], bf16, kind="ExternalInput").ap()
    bp0_d = nc.dram_tensor("bp0", [128, N + 1, G], bf16, kind="ExternalInput").ap()
    bp1_d = nc.dram_tensor("bp1", [64, N + 1, G], bf16, kind="ExternalInput").ap()
    rw1_d = nc.dram_tensor("rw1", [128, 3, KT, HS], bf16, kind="ExternalInput").ap()
    rw2_d = nc.dram_tensor("rw2", [HS, 100], bf16, kind="ExternalInput").ap()
    rw3_d = nc.dram_tensor("rw3", [100, 11], bf16, kind="ExternalInput").ap()
    rb3_d = nc.dram_tensor("rb3", [11, 1], f32, kind="ExternalInput").ap()
    sel_d = nc.dram_tensor("sel", [11, 11, 128], f32r, kind="ExternalInput").ap()
    ones_d = nc.dram_tensor("onesd", [1, 128], f32r, kind="ExternalInput").ap()
    onesb_d = nc.dram_tensor("onesb16", [128, 2], bf16, kind="ExternalInput").ap()
    outT_d = nc.dram_tensor("outT", [DS, B], bf16, kind="ExternalOutput").ap()

    pools = {}

    def pool(name, bufs, space="SBUF"):
        cm = tc.tile_pool(name=name, bufs=bufs, space=space)
        pools[name] = cm
        return cm.__enter__()

    cp = pool("const", 1)       # persistent constants / accumulators
    xtp = pool("xt", 3)         # [128, KT, B] bf16 whole-group x
    wtp = pool("wt", 2)         # [128, KH, G, DS] bf16 half-group W
    wsp = pool("ws", 2)         # [128, KT, DS] bf16 e-reduced weights
    wrp = pool("wred", 3)       # [128, KH, DS] bf16 e-reduce tree temps
    gs0 = pool("gs0", 8)        # [128, B] bf16 staged raw (chunk 0)
    gs1 = pool("gs1", 8)        # [64, B] bf16 staged raw (chunk 1)
    bcp = pool("bc", 2)         # [128, B] bf16 staged gate broadcast
    b1k = pool("big1k", 2)      # [128, B] f32 scratch (bn dumps)
    ctb = pool("ctb", 4)        # [128, B] bf16 scratch (squares/folds)
    ctp = pool("ctmp", 3)       # [128, BSL] f32 scratch
    stp = pool("stat", 10)      # [128, 1] BN stats smalls
    psp = pool("ps", 8, space="PSUM")
    drp = pool("dram", 1, space="DRAM")

    def ps_tile(p, n=BSL):
        return psp.tile([p, n], f32, tag="ps", name="ps")

    # ---- constants (gpsimd SWDGE queue; sync queue kept clear for x/W) ----
    ones1 = cp.tile([1, 128], f32r, tag="ones1", name="ones1")
    nc.gpsimd.dma_start(ones1[:], ones_d[:])
    onesb = cp.tile([128, 1], bf16, tag="onesb", name="onesb")
    nc.gpsimd.dma_start(onesb[:], onesb_d[:, 0:1])
    twosb = cp.tile([128, 1], bf16, tag="twosb", name="twosb")
    nc.gpsimd.dma_start(twosb[:], onesb_d[:, 1:2])
    sel = cp.tile([11, 11, 128], f32r, tag="sel", name="sel")
    nc.gpsimd.dma_start(sel[:], sel_d[:])
    rb3 = cp.tile([11, 1], f32, tag="rb3", name="rb3")
    nc.gpsimd.dma_start(rb3[:], rb3_d[:])
    rw2 = cp.tile([HS, 100], bf16, tag="rw2", name="rw2")
    nc.gpsimd.dma_start(rw2[:], rw2_d[:])
    rw3 = cp.tile([100, 11], bf16, tag="rw3", name="rw3")
    nc.gpsimd.dma_start(rw3[:], rw3_d[:])
    bp0 = cp.tile([128, N + 1, G], bf16, tag="bp0", name="bp0")
    nc.gpsimd.dma_start(bp0[:], bp0_d[:])
    bp1 = cp.tile([64, N + 1, G], bf16, tag="bp1", name="bp1")
    nc.gpsimd.dma_start(bp1[:], bp1_d[:])

    # rw1 slice first on the sync queue: h1 matmuls only need rt + xT[0]
    rt = cp.tile([128, 3, KT, HS], bf16, tag="rt", name="rt")
    nc.sync.dma_start(rt[:], rw1_d[:])

    # bias e-reduce: [p, 11, G] -> [p, 11] f32 columns
    bsum0 = cp.tile([128, N + 1], f32, tag="bsum0", name="bsum0")
    nc.vector.reduce_sum(out=bsum0[:], in_=bp0[:], axis=AX.X)
    bsum1 = cp.tile([64, N + 1], f32, tag="bsum1", name="bsum1")
    nc.vector.reduce_sum(out=bsum1[:], in_=bp1[:], axis=AX.X)
    bsums = (bsum0, bsum1)
    # uni bias column pre-scaled by 0.1/7
    bib_sc = []
    for ci, (m0, msz) in enumerate(CHUNKS):
        c2 = stp.tile([128, 1], f32, tag="stat", name="stat")
        nc.vector.tensor_scalar_mul(c2[0:msz, :], bsums[ci][0:msz, N:N + 1],
                                    0.1 / 7.0)
        bib_sc.append(c2)

    # ---- persistent buffers ----
    h1loc = cp.tile([HS, B], f32, tag="h1loc", name="h1loc")
    h2g = cp.tile([100, B], bf16, tag="h2g", name="h2g")
    A = [cp.tile([msz, B], bf16, tag=f"A{ci}", name=f"A{ci}")
         for ci, (m0, msz) in enumerate(CHUNKS)]
    z = [cp.tile([msz, B], bf16, tag=f"z{ci}", name=f"z{ci}")
         for ci, (m0, msz) in enumerate(CHUNKS)]
    outsb = [cp.tile([msz, B], bf16, tag=f"out{ci}", name=f"out{ci}")
             for ci, (m0, msz) in enumerate(CHUNKS)]
    h2b = cp.tile([100, B], bf16, tag="h2b", name="h2b")
    ex10 = cp.tile([11, B], f32r, tag="ex10", name="ex10")
    arv = cp.tile([1, 3 * B], f32, tag="arv", name="arv")
    tv = cp.tile([1, B], f32r, tag="tv", name="tv")
    uv = cp.tile([1, B], f32r, tag="uv", name="uv")
    xu = []
    for ci, (m0, msz) in enumerate(CHUNKS):
        xu.append(cp.tile([msz, B], bf16, tag=f"xu{ci}", name=f"xu{ci}"))

    cc1_in = drp.tile([1, B], f32, tag="cc1_in", name="cc1_in")
    # AR1 result laid out [128, 8] so the 1/||U|| reciprocal runs 128-wide
    # (a [1, B] row reciprocal is a 6.5us single-partition stall that Tile
    # schedules mid-pipeline on the DVE queue, starving the PE)
    cc1_out = drp.tile([128, B // 128], f32, tag="cc1_out",
                       addr_space="Shared", name="cc1_out")
    us_dram = drp.tile([1, B], f32r, tag="us_dram", name="us_dram")
    cc2_in = drp.tile([1, B], f32, tag="cc2_in", name="cc2_in")
    # AR2 result laid out [128, 8] so the tail rsqrt runs 128-wide
    cc2_out = drp.tile([128, B // 128], f32, tag="cc2_out",
                       addr_space="Shared", name="cc2_out")
    cc3_in = drp.tile([100, B], bf16, tag="cc3_in", name="cc3_in")
    cc3_out = drp.tile([100, B], bf16, tag="cc3_out",
                       addr_space="Shared", name="cc3_out")
    u_dram = drp.tile([1, B], f32r, tag="u_dram", name="u_dram")

    def stream_group(xsrc, wsrc):
        """DMA one group's x (one shot) + W (two halves), reduce W over e.

        The e-reduction is a pairwise add tree with expert slices contiguous
        along DS (step-1 bf16 operands hit the DVE 2x mode).
        """
        xt = xtp.tile([128, KT, B], bf16, tag="xt", name="xt")
        nc.sync.dma_start(xt[:], xsrc)
        ws = wsp.tile([128, KT, DS], bf16, tag="ws", name="ws")
        for h in range(2):
            wt = wtp.tile([128, KH, G, DS], bf16, tag="wt", name="wt")
            nc.sync.dma_start(wt[:], wsrc[:, h * KH:(h + 1) * KH, :, :])
            t01 = wrp.tile([128, KH, DS], bf16, tag="wred", name="wred")
            nc.vector.tensor_tensor(out=t01[:], in0=wt[:, :, 0, :],
                                    in1=wt[:, :, 1, :], op=ALU.add)
            t23 = wrp.tile([128, KH, DS], bf16, tag="wred", name="wred")
            nc.vector.tensor_tensor(out=t23[:], in0=wt[:, :, 2, :],
                                    in1=wt[:, :, 3, :], op=ALU.add)
            t45 = wrp.tile([128, KH, DS], bf16, tag="wred", name="wred")
            nc.vector.tensor_tensor(out=t45[:], in0=wt[:, :, 4, :],
                                    in1=wt[:, :, 5, :], op=ALU.add)
            nc.vector.tensor_tensor(out=t01[:], in0=t01[:], in1=t23[:],
                                    op=ALU.add)
            nc.vector.tensor_tensor(out=t45[:], in0=t45[:], in1=wt[:, :, 6, :],
                                    op=ALU.add)
            nc.vector.tensor_tensor(out=ws[:, h * KH:(h + 1) * KH, :],
                                    in0=t01[:], in1=t45[:], op=ALU.add)
        return xt, ws

    def group_matmuls(xt, ws, g):
        """raw = x @ Wsum (+ e-summed bias on the ACT copy); stage bf16."""
        raws = []
        for ci, (m0, msz) in enumerate(CHUNKS):
            gp = gs0 if ci == 0 else gs1
            raw = gp.tile([msz, B], bf16, tag=f"gs{ci}", name=f"gs{ci}")
            for bs in range(NBS):
                sl = slice(bs * BSL, (bs + 1) * BSL)
                ps = ps_tile(msz)
                for kt in range(KT):
                    nc.tensor.matmul(
                        ps[:],
                        lhsT=ws[:, kt, m0:m0 + msz],
                        rhs=xt[:, kt, sl],
                        start=(kt == 0), stop=(kt == KT - 1))
                nc.scalar.activation(raw[:, sl], ps[:], ACTF.Identity,
                                     bias=bsums[ci][0:msz, g:g + 1])
            raws.append(raw)
        return raws

    def h1_mms(ri, xt):
        """Local h1 partial for router group ri: [HS, B] over this core's
        64 neurons (contraction over this group's 1024 k)."""
        for bs in range(NBS):
            sl = slice(bs * BSL, (bs + 1) * BSL)
            ps = ps_tile(HS)
            for kt in range(KT):
                nc.tensor.matmul(
                    ps[:],
                    lhsT=rt[:, ri, kt, :],
                    rhs=xt[:, kt, sl],
                    start=(kt == 0), stop=(kt == KT - 1))
            if ri == 0:
                nc.scalar.copy(h1loc[:, sl], ps[:])
            else:
                nc.vector.tensor_tensor(out=h1loc[:, sl], in0=ps[:],
                                        in1=h1loc[:, sl], op=ALU.add)

    fold_state = {"first": True}

    def fold_group(raws, i):
        """A += e_i * raw. Gate row i is PE-broadcast to 128 partitions,
        ACT-staged to SBUF bf16 so the DVE folds run in 2x mode."""
        bcs = bcp.tile([128, B], bf16, tag="bc", name="bc")
        for bs in range(NBS):
            sl = slice(bs * BSL, (bs + 1) * BSL)
            bc = ps_tile(128)
            nc.tensor.matmul(bc[:], lhsT=sel[:, i, :],
                             rhs=ex10[:, sl], start=True, stop=True)
            nc.scalar.copy(bcs[:, sl], bc[:])
        first = fold_state["first"]
        fold_state["first"] = False
        for ci, (m0, msz) in enumerate(CHUNKS):
            if first:
                nc.vector.tensor_tensor(out=A[ci][:], in0=raws[ci][:],
                                        in1=bcs[0:msz, :], op=ALU.mult)
            else:
                tmp = ctb.tile([128, B], bf16, tag="ctb", name="ctb")
                nc.vector.tensor_tensor(out=tmp[0:msz, :], in0=raws[ci][:],
                                        in1=bcs[0:msz, :], op=ALU.mult)
                nc.vector.tensor_tensor(out=A[ci][:], in0=A[ci][:],
                                        in1=tmp[0:msz, :], op=ALU.add)

    def bn_act(tiles, out_tiles, nparts, func):
        """BatchNorm (training stats over free axis) + activation."""
        for t, to in zip(tiles, out_tiles):
            dump = b1k.tile([128, B], f32, tag="big1k", name="big1k")
            mnr = stp.tile([128, 1], f32, tag="stat", name="stat")
            nc.scalar.activation(dump[0:nparts, :], t[:], ACTF.Copy,
                                 accum_out=mnr[0:nparts, :])
            mn = stp.tile([128, 1], f32, tag="stat", name="stat")
            nc.scalar.mul(mn[0:nparts, :], mnr[0:nparts, :], 1.0 / B)
            sq = b1k.tile([128, B], f32, tag="big1k", name="big1k")
            ex2r = stp.tile([128, 1], f32, tag="stat", name="stat")
            nc.scalar.activation(sq[0:nparts, :], t[:], ACTF.Square,
                                 accum_out=ex2r[0:nparts, :])
            ex2 = stp.tile([128, 1], f32, tag="stat", name="stat")
            nc.scalar.mul(ex2[0:nparts, :], ex2r[0:nparts, :], 1.0 / B)
            var = stp.tile([128, 1], f32, tag="stat", name="stat")
            nc.vector.tensor_tensor(out=var[0:nparts, :], in0=mn[0:nparts, :],
                                    in1=mn[0:nparts, :], op=ALU.mult)
            nc.vector.tensor_tensor(out=var[0:nparts, :], in0=ex2[0:nparts, :],
                                    in1=var[0:nparts, :], op=ALU.subtract)
            nc.vector.tensor_scalar_add(var[0:nparts, :], var[0:nparts, :],
                                        EPS_BN)
            sd = stp.tile([128, 1], f32, tag="stat", name="stat")
            nc.scalar.sqrt(sd[0:nparts, :], var[0:nparts, :])
            rs = stp.tile([128, 1], f32, tag="stat", name="stat")
            nc.vector.reciprocal(rs[0:nparts, :], sd[0:nparts, :])
            nb = stp.tile([128, 1], f32, tag="stat", name="stat")
            nc.vector.tensor_tensor(out=nb[0:nparts, :], in0=mn[0:nparts, :],
                                    in1=rs[0:nparts, :], op=ALU.mult)
            nc.vector.tensor_scalar_mul(nb[0:nparts, :], nb[0:nparts, :], -1.0)
            nc.scalar.activation(to[:], t[:], func,
                                 bias=nb[0:nparts, :], scale=rs[0:nparts, :])

    # ================= router groups first (PE starts on h1 at ~8us) ======
    router_raws = {}
    for ri, g in enumerate(ROUTER_GS):
        xt, ws = stream_group(xT_d[g], w_d[g])
        h1_mms(ri, xt)
        router_raws[g] = group_matmuls(xt, ws, g)

    # bn+relu on local h1 slice, then this core's h2 partial through its
    # 64-row slice of rw2; the cross-core coupling is a small [100, B]
    # bf16 AllReduce instead of a 1MB h1 AllGather (bcp pool is not
    # contended until the folds start, long after this is consumed)
    h1b_loc = bcp.tile([128, B], bf16, tag="bc", name="bc")
    bn_act([h1loc], [h1b_loc[0:HS, :]], HS, ACTF.Relu)
    h2p = ctb.tile([128, B], bf16, tag="ctb", name="ctb")
    for bs in range(NBS):
        sl = slice(bs * BSL, (bs + 1) * BSL)
        ps = ps_tile(100)
        nc.tensor.matmul(ps[:], lhsT=rw2[:], rhs=h1b_loc[0:HS, sl],
                         start=True, stop=True)
        nc.scalar.copy(h2p[0:100, sl], ps[:])
    nc.sync.dma_start(cc3_in[:], h2p[0:100, :])
    nc.gpsimd.collective_compute(
        "AllReduce", ALU.add,
        ins=[cc3_in.opt()], outs=[cc3_out.opt()],
        replica_groups=[list(range(NCORES))])

    # ================= uni branch -> z, ssz partial, AR1 =================
    for ci, (m0, msz) in enumerate(CHUNKS):
        nc.sync.dma_start(xu[ci][:], xuT_d[m0:m0 + msz, :])
    xib_t, wib_s = stream_group(xibT_d, wib_d)
    for ci, (m0, msz) in enumerate(CHUNKS):
        for bs in range(NBS):
            sl = slice(bs * BSL, (bs + 1) * BSL)
            ps = ps_tile(msz)
            for kt in range(KT):
                nc.tensor.matmul(ps[:], lhsT=wib_s[:, kt, m0:m0 + msz],
                                 rhs=xib_t[:, kt, sl],
                                 start=(kt == 0), stop=(kt == KT - 1))
            # z = (raw + bias)*0.1/7 + 0.9*xu  (fused add on DVE)
            nc.scalar.activation(z[ci][:, sl], ps[:], ACTF.Identity,
                                 scale=0.1 / 7.0, bias=bib_sc[ci][0:msz, :])
            nc.vector.scalar_tensor_tensor(
                out=z[ci][:, sl], in0=xu[ci][:, sl], scalar=0.9,
                in1=z[ci][:, sl], op0=ALU.mult, op1=ALU.add)
    # ssz partial: sum_d z^2 -> arv slot 2 -> cc1_in
    for bs in range(NBS):
        sl = slice(bs * BSL, (bs + 1) * BSL)
        ps = ps_tile(1)
        for ci, (m0, msz) in enumerate(CHUNKS):
            sq = ctb.tile([128, B], bf16, tag="ctb", name="ctb")
            nc.scalar.square(sq[0:msz, 0:BSL], z[ci][:, sl])
            nc.tensor.matmul(ps[:], lhsT=onesb[0:msz, :],
                             rhs=sq[0:msz, 0:BSL],
                             start=(ci == 0), stop=(ci == len(CHUNKS) - 1))
        nc.scalar.copy(arv[:, B + bs * BSL:B + (bs + 1) * BSL], ps[:])
    nc.sync.dma_start(cc1_in[:], arv[:, B:2 * B])
    nc.gpsimd.collective_compute(
        "AllReduce", ALU.add,
        ins=[cc1_in.opt()], outs=[cc1_out.opt()],
        replica_groups=[list(range(NCORES))])

    # ===== stream groups 1,2,3,5 (PE cover for bn/AG/finalize latency) ====
    pre_raws = {}
    for g in (1, 2, 3):
        xt, ws = stream_group(xT_d[g], w_d[g])
        pre_raws[g] = group_matmuls(xt, ws, g)

    # ================= router finalize =================
    # h2 AllReduce result fetched on the gpsimd queue: its completion wait
    # must not head-of-line-block the sync (x/W streams) or scalar (ACT
    # staging) queues, and gpsimd has nothing urgent queued behind it.
    nc.gpsimd.dma_start(h2g[:], cc3_out[:])
    g5_xt, g5_ws = stream_group(xT_d[5], w_d[5])
    pre_raws[5] = group_matmuls(g5_xt, g5_ws, 5)
    bn_act([h2g], [h2b], 100, ACTF.Tanh)
    for bs in range(NBS):
        sl = slice(bs * BSL, (bs + 1) * BSL)
        ps = ps_tile(11)
        nc.tensor.matmul(ps[:], lhsT=rw3[:],
                         rhs=h2b[:, sl], start=True, stop=True)
        sg = ctp.tile([128, BSL], f32, tag="ctmp", name="ctmp")
        nc.scalar.activation(sg[0:11, :], ps[:], ACTF.Sigmoid, bias=rb3[:],
                             scale=1.0)
        nc.scalar.activation(ex10[:, sl], sg[0:11, :], ACTF.Exp, scale=10.0)

    # e10 row (no AR1 dependency): arv slot 3 <- 7*e10
    for bs in range(NBS):
        sl = slice(bs * BSL, (bs + 1) * BSL)
        ps = ps_tile(1)
        nc.tensor.matmul(ps[:], lhsT=sel[:, 10, 0:1],
                         rhs=ex10[:, sl], start=True, stop=True)
        nc.scalar.copy(arv[:, 2 * B + bs * BSL:2 * B + (bs + 1) * BSL], ps[:])
    e10v = arv[:, 2 * B:3 * B]
    nc.vector.tensor_scalar_mul(e10v, e10v, 7.0)

    # ================= folds: staged groups, then stream the rest =========
    for g in (0, 4, 8, 1, 2, 3):
        fold_group(router_raws.get(g) or pre_raws[g], g)
    fold_group(pre_raws[5], 5)

    for g in (6, 7, 9):
        xt, ws = stream_group(xT_d[g], w_d[g])
        raws = group_matmuls(xt, ws, g)
        fold_group(raws, g)

    # AR1 result -> 1/||U|| on [128, 8] (cheap 128-wide recip), bounced
    # through DRAM back into [1, B] row form for the gate-row arithmetic.
    # All on the gpsimd queue, which has nothing urgent behind this.
    s8 = cp.tile([128, B // 128], f32, tag="s8", name="s8")
    nc.gpsimd.dma_start(s8[:], cc1_out[:])
    nc.scalar.sqrt(s8[:], s8[:])
    nc.vector.tensor_scalar_max(s8[:], s8[:], EPS_NORM)
    s8r = cp.tile([128, B // 128], f32r, tag="s8r", name="s8r")
    nc.vector.reciprocal(s8r[:], s8[:])
    nc.gpsimd.dma_start(us_dram[:], s8r[:])
    nc.gpsimd.dma_start(uv[:], us_dram[:])
    nc.vector.tensor_tensor(out=tv[:], in0=e10v, in1=uv[:], op=ALU.mult)
    # e10sq/8: each core contributes 1/8 so the AR2 sum restores it
    nc.vector.tensor_tensor(out=e10v, in0=e10v, in1=e10v, op=ALU.mult)
    nc.vector.tensor_scalar_mul(e10v, e10v, 0.125)

    # ========= tail: q_loc = |A|^2 + t*(2 A.z) + (7e10)^2/8 -> AR2 ========
    for bs in range(NBS):
        sl = slice(bs * BSL, (bs + 1) * BSL)
        psa = ps_tile(1)
        psc = ps_tile(1)
        for ci, (m0, msz) in enumerate(CHUNKS):
            sqa = ctb.tile([128, B], bf16, tag="ctb", name="ctb")
            nc.scalar.square(sqa[0:msz, 0:BSL], A[ci][:, sl])
            nc.tensor.matmul(psa[:], lhsT=onesb[0:msz, :],
                             rhs=sqa[0:msz, 0:BSL],
                             start=(ci == 0), stop=(ci == len(CHUNKS) - 1))
            cza = ctb.tile([128, B], bf16, tag="ctb", name="ctb")
            nc.vector.tensor_tensor(out=cza[0:msz, 0:BSL], in0=A[ci][:, sl],
                                    in1=z[ci][:, sl], op=ALU.mult)
            nc.tensor.matmul(psc[:], lhsT=twosb[0:msz, :],
                             rhs=cza[0:msz, 0:BSL],
                             start=(ci == 0), stop=(ci == len(CHUNKS) - 1))
        qt = ctp.tile([128, BSL], f32, tag="ctmp", name="ctmp")
        nc.vector.tensor_tensor(out=qt[0:1, :], in0=psc[:], in1=tv[:, sl],
                                op=ALU.mult)
        nc.vector.tensor_tensor(out=qt[0:1, :], in0=qt[0:1, :], in1=psa[:],
                                op=ALU.add)
        nc.vector.tensor_tensor(out=arv[:, bs * BSL:(bs + 1) * BSL],
                                in0=qt[0:1, :],
                                in1=arv[:, 2 * B + bs * BSL:2 * B + (bs + 1) * BSL],
                                op=ALU.add)
    nc.sync.dma_start(cc2_in[:], arv[:, 0:B])
    nc.gpsimd.collective_compute(
        "AllReduce", ALU.add,
        ins=[cc2_in.opt()], outs=[cc2_out.opt()],
        replica_groups=[list(range(NCORES))])

    # P = A + t*z, computed while AR2 is in flight
    tvb = bcp.tile([128, B], bf16, tag="bc", name="bc")
    for bs in range(NBS):
        sl = slice(bs * BSL, (bs + 1) * BSL)
        btv = ps_tile(128)
        nc.tensor.matmul(btv[:], lhsT=ones1[:],
                         rhs=tv[:, sl], start=True, stop=True)
        nc.scalar.copy(tvb[:, sl], btv[:])
    for ci, (m0, msz) in enumerate(CHUNKS):
        tmp = ctb.tile([128, B], bf16, tag="ctb", name="ctb")
        nc.vector.tensor_tensor(out=tmp[0:msz, :], in0=z[ci][:],
                                in1=tvb[0:msz, :], op=ALU.mult)
        nc.vector.tensor_tensor(out=A[ci][:], in0=A[ci][:],
                                in1=tmp[0:msz, :], op=ALU.add)

    # sqrt ACT table preload during AR2 flight: the tail sqrt otherwise
    # pays a ~1.5us ACT_TABLE_LOAD on the critical path
    dwarm = stp.tile([128, 1], f32, tag="stat", name="stat")
    nc.scalar.sqrt(dwarm[0:1, :], bib_sc[0][0:1, :])

    # q -> u = 1/max(sqrt(q), eps) on [128, 8] (a [1, B] row reciprocal is
    # a 7.8us single-partition iterative divide; 128-wide it is ~0.1us),
    # then a DRAM bounce turns u back into a [1, B] row for the broadcast
    u8 = cp.tile([128, B // 128], f32, tag="u8", name="u8")
    nc.sync.dma_start(u8[:], cc2_out[:])
    nc.scalar.sqrt(u8[:], u8[:])
    nc.vector.tensor_scalar_max(u8[:], u8[:], EPS_NORM)
    u8r = cp.tile([128, B // 128], f32r, tag="u8r", name="u8r")
    nc.vector.reciprocal(u8r[:], u8[:])
    nc.sync.dma_start(u_dram[:], u8r[:])
    ubrow = cp.tile([1, B], f32r, tag="ubrow", name="ubrow")
    nc.sync.dma_start(ubrow[:], u_dram[:])
    ub = bcp.tile([128, B], bf16, tag="bc", name="bc")
    for bs in range(NBS):
        sl = slice(bs * BSL, (bs + 1) * BSL)
        bu = ps_tile(128)
        nc.tensor.matmul(bu[:], lhsT=ones1[:],
                         rhs=ubrow[:, sl], start=True, stop=True)
        nc.scalar.copy(ub[:, sl], bu[:])
    for ci, (m0, msz) in enumerate(CHUNKS):
        nc.vector.tensor_tensor(out=outsb[ci][:], in0=A[ci][:],
                                in1=ub[0:msz, :], op=ALU.mult)
        nc.sync.dma_start(outT_d[m0:m0 + msz, :], outsb[ci][:])

    for p in reversed(list(pools.values())):
        p.__exit__(None, None, None)


def _build_nc():
    nc = bacc.Bacc("TRN2", target_bir_lowering=False, debug=False,
                   num_devices=NCORES)
    with tile.TileContext(nc) as tc:
        with nc.allow_low_precision(reason="bf16 streams / f32r reductions are intentional"):
            _emit(nc, tc)
    nc.compile()
    return nc


def _as_bf16(a):
    return np.ascontiguousarray(a.astype(ml_dtypes.bfloat16))


def _host_prep(inputs):
    x_enc = np.asarray(inputs["x_enc"], dtype=np.float32)
    x_ib = np.asarray(inputs["x_ib"], dtype=np.float32)
    x_uni = np.asarray(inputs["x_uni"], dtype=np.float32)
    W_proj = np.asarray(inputs["W_proj"], dtype=np.float32)
    b_proj = np.asarray(inputs["b_proj"], dtype=np.float32)
    W_ib = np.asarray(inputs["W_ib"], dtype=np.float32)
    b_ib = np.asarray(inputs["b_ib"], dtype=np.float32)

    # x_enc [N,B,K] -> [N, 128, KT, B] partition-major bf16
    xT = _as_bf16(x_enc.transpose(0, 2, 1).reshape(N, KT, 128, B).transpose(0, 2, 1, 3))
    # x_ib [B,K] -> [128, KT, B]
    xibT = _as_bf16(x_ib.T.reshape(KT, 128, B).transpose(1, 0, 2))
    sel = np.zeros((11, 11, 128), dtype=np.float32)
    for q in range(11):
        sel[q, q, :] = 1.0
    rb3 = np.ascontiguousarray(np.asarray(inputs["r_b3"], np.float32).reshape(11, 1))
    # r_w1 [3072, 512]: per-core h-slice -> [128, 3, KT, 64]
    rw1_full = np.asarray(inputs["r_w1"], np.float32)
    rw2_full = np.asarray(inputs["r_w2"], np.float32)
    rw3 = _as_bf16(np.asarray(inputs["r_w3"], np.float32))
    ones_host = np.ones((1, 128), dtype=np.float32)
    onesb16 = np.ones((128, 2), dtype=np.float32)
    onesb16[:, 1] = 2.0
    onesb16 = _as_bf16(onesb16)

    in_maps = []
    for c in range(NCORES):
        ds = slice(c * DS, (c + 1) * DS)
        # W_proj [N,G,K,D] ds-slice -> [N, 128, KT, G, DS]
        wc = _as_bf16(W_proj[:, :, :, ds].reshape(N, G, KT, 128, DS)
                      .transpose(0, 3, 2, 1, 4))
        wibc = _as_bf16(W_ib[:, :, ds].reshape(G, KT, 128, DS)
                        .transpose(2, 1, 0, 3))
        # biases: [DS, 11, G] with group 10 = b_ib, split into chunks
        bp_full = np.zeros((DS, N + 1, G), dtype=np.float32)
        bp_full[:, :N, :] = b_proj[:, :, ds].transpose(2, 0, 1)
        bp_full[:, N, :] = b_ib[:, ds].T
        bp_full = _as_bf16(bp_full)
        rw1c = _as_bf16(rw1_full[:, c * HS:(c + 1) * HS]
                        .reshape(3, KT, 128, HS).transpose(2, 0, 1, 3))
        rw2c = _as_bf16(rw2_full[c * HS:(c + 1) * HS, :])
        in_maps.append({
            "xT": xT,
            "xibT": xibT,
            "w": wc,
            "wib": wibc,
            "xuT": _as_bf16(x_uni[:, ds].T),
            "bp0": bp_full[0:128],
            "bp1": bp_full[128:192],
            "rw1": rw1c,
            "rw2": rw2c,
            "rw3": rw3,
            "rb3": rb3,
            "sel": sel,
            "onesd": ones_host,
            "onesb16": onesb16,
        })
    return in_maps


def kernel(**inputs):
    global LAST_RESULTS
    if "nc" not in _NC_CACHE:
        _NC_CACHE["nc"] = _build_nc()
    nc = _NC_CACHE["nc"]
    in_maps = _host_prep(inputs)
    res = run_bass_kernel_spmd(nc, in_maps, list(range(NCORES)))
    LAST_RESULTS = res
    full = np.concatenate(
        [res.results[c]["outT"].astype(np.float32) for c in range(NCORES)],
        axis=0)
    return np.ascontiguousarray(full.T)


# revision 22
# speedup vs baseline: 1.1647x; 1.1214x over previous
"""Trainium2 Bass kernel for nn_ATVP_router_UNI (moe_routing).

Sharding: output dim D=1536 sharded over 8 cores (192 each). Activations
(x_enc, x_ib) are broadcast in a partition-major transposed layout. The
router's first layer (3072->512) is h-neuron-sharded across cores (64
neurons each; BatchNorm stats are per-neuron so they stay local) with an
AllGather of the post-ReLU bf16 activations. The expert mean over e
commutes with the linear projection, so each core streams its W_proj
slice once (bf16), reduces over e on DVE, and runs 1/7 of the naive
matmul FLOPs in bf16. The softmax denominator and the 1/7 group-mean
scale cancel inside the final l2 normalization and are folded away.
Per-expert biases are batch-loaded, e-reduced on DVE, folded into the
PSUM->SBUF copy (ACT Identity bias).

Scheduling (PE is the measured bottleneck): router group 0 streams
first so the PE starts at ~8us; the uni branch runs after the router
groups; group streams are interleaved with the router finalize so the
PE FIFO never head-of-line blocks on BN; the gated folds run in bf16
against an SBUF-staged gate broadcast (2x DVE mode); the A accumulator
and all tail intermediates are bf16. Cross-core coupling: AllGather of
h1 (hidden under streaming), AllReduce of the uni sum-of-squares
(hidden), and the 4KB out-norm AllReduce on the tail (the t*z fold
runs during its flight).

kernel(**inputs) takes the full unsharded inputs and returns the full
[1024, 1536] f32 output. Host-side prep does layout/dtype staging only --
all arithmetic runs on device.
"""

import numpy as np
import ml_dtypes

import concourse.bass as bass
import concourse.tile as tile
import concourse.mybir as mybir
from concourse import bacc
from concourse.bass_utils import run_bass_kernel_spmd

f32 = mybir.dt.float32
f32r = mybir.dt.float32r
bf16 = mybir.dt.bfloat16
AX = mybir.AxisListType
ALU = mybir.AluOpType
ACTF = mybir.ActivationFunctionType

NCORES = 8
B, N, G, K, D = 1024, 10, 7, 1024, 1536
DS = D // NCORES          # 192 output dims per core
KT = K // 128             # 8 k-tiles
KH = KT // 2              # 4 k-tiles per W half-transfer
BSL = 512                 # free-dim slice for matmuls (PSUM bank limit)
NBS = B // BSL            # 2
HS = 512 // NCORES        # 64 router h1 neurons per core
ROUTER_GS = (0, 4, 8)     # TEXT_PRED groups, streamed first
CHUNKS = ((0, 128), (128, 64))  # (d-offset, size) chunks of DS=192
EPS_BN = 1e-5
EPS_NORM = 1e-12

LAST_RESULTS = None
_NC_CACHE = {}


def _emit(nc, tc):
    # ---- DRAM I/O ----
    xT_d = nc.dram_tensor("xT", [N, 128, KT, B], bf16, kind="ExternalInput").ap()
    xibT_d = nc.dram_tensor("xibT", [128, KT, B], bf16, kind="ExternalInput").ap()
    w_d = nc.dram_tensor("w", [N, 128, KT, G, DS], bf16, kind="ExternalInput").ap()
    wib_d = nc.dram_tensor("wib", [128, KT, G, DS], bf16, kind="ExternalInput").ap()
    xuT_d = nc.dram_tensor("xuT", [DS, B], bf16, kind="ExternalInput").ap()
    bp0_d = nc.dram_tensor("bp0", [128, N + 1, G], bf16, kind="ExternalInput").ap()
    bp1_d = nc.dram_tensor("bp1", [64, N + 1, G], bf16, kind="ExternalInput").ap()
    rw1_d = nc.dram_tensor("rw1", [128, 3, KT, HS], bf16, kind="ExternalInput").ap()
    rw2_d = nc.dram_tensor("rw2", [HS, 100], bf16, kind="ExternalInput").ap()
    rw3_d = nc.dram_tensor("rw3", [100, 11], bf16, kind="ExternalInput").ap()
    rb3_d = nc.dram_tensor("rb3", [11, 1], f32, kind="ExternalInput").ap()
    sel_d = nc.dram_tensor("sel", [11, 11, 128], f32r, kind="ExternalInput").ap()
    ones_d = nc.dram_tensor("onesd", [1, 128], f32r, kind="ExternalInput").ap()
    onesb_d = nc.dram_tensor("onesb16", [128, 2], bf16, kind="ExternalInput").ap()
    outT_d = nc.dram_tensor("outT", [DS, B], bf16, kind="ExternalOutput").ap()

    pools = {}

    def pool(name, bufs, space="SBUF"):
        cm = tc.tile_pool(name=name, bufs=bufs, space=space)
        pools[name] = cm
        return cm.__enter__()

    cp = pool("const", 1)       # persistent constants / accumulators
    xtp = pool("xt", 2)         # [128, KT, B] bf16 whole-group x
    wtp = pool("wt", 2)         # [128, KH, G, DS] bf16 half-group W
    wsp = pool("ws", 2)         # [128, KT, DS] bf16 e-reduced weights
    wrp = pool("wred", 3)       # [128, KH, DS] bf16 e-reduce tree temps
    # one staged-raw buffer per group: a late router AllReduce (the fold
    # gate) must never backpressure the W/x streaming pipeline
    gs0 = pool("gs0", 11)       # [128, B] bf16 staged raw (chunk 0)
    gs1 = pool("gs1", 11)       # [64, B] bf16 staged raw (chunk 1)
    bcp = pool("bc", 2)         # [128, B] bf16 staged gate broadcast
    b1k = pool("big1k", 2)      # [128, B] f32 scratch (bn dumps)
    ctb = pool("ctb", 4)        # [128, B] bf16 scratch (squares/folds)
    ctp = pool("ctmp", 3)       # [128, BSL] f32 scratch
    stp = pool("stat", 10)      # [128, 1] BN stats smalls
    psp = pool("ps", 8, space="PSUM")
    drp = pool("dram", 1, space="DRAM")

    def ps_tile(p, n=BSL):
        return psp.tile([p, n], f32, tag="ps", name="ps")

    # ---- constants (gpsimd SWDGE queue; sync queue kept clear for x/W) ----
    ones1 = cp.tile([1, 128], f32r, tag="ones1", name="ones1")
    nc.gpsimd.dma_start(ones1[:], ones_d[:])
    onesb = cp.tile([128, 1], bf16, tag="onesb", name="onesb")
    nc.gpsimd.dma_start(onesb[:], onesb_d[:, 0:1])
    twosb = cp.tile([128, 1], bf16, tag="twosb", name="twosb")
    nc.gpsimd.dma_start(twosb[:], onesb_d[:, 1:2])
    sel = cp.tile([11, 11, 128], f32r, tag="sel", name="sel")
    nc.gpsimd.dma_start(sel[:], sel_d[:])
    rb3 = cp.tile([11, 1], f32, tag="rb3", name="rb3")
    nc.gpsimd.dma_start(rb3[:], rb3_d[:])
    rw2 = cp.tile([HS, 100], bf16, tag="rw2", name="rw2")
    nc.gpsimd.dma_start(rw2[:], rw2_d[:])
    rw3 = cp.tile([100, 11], bf16, tag="rw3", name="rw3")
    nc.gpsimd.dma_start(rw3[:], rw3_d[:])
    bp0 = cp.tile([128, N + 1, G], bf16, tag="bp0", name="bp0")
    nc.gpsimd.dma_start(bp0[:], bp0_d[:])
    bp1 = cp.tile([64, N + 1, G], bf16, tag="bp1", name="bp1")
    nc.gpsimd.dma_start(bp1[:], bp1_d[:])

    # rw1 slice first on the sync queue: h1 matmuls only need rt + xT[0]
    rt = cp.tile([128, 3, KT, HS], bf16, tag="rt", name="rt")
    nc.sync.dma_start(rt[:], rw1_d[:])

    # bias e-reduce: [p, 11, G] -> [p, 11] f32 columns
    bsum0 = cp.tile([128, N + 1], f32, tag="bsum0", name="bsum0")
    nc.vector.reduce_sum(out=bsum0[:], in_=bp0[:], axis=AX.X)
    bsum1 = cp.tile([64, N + 1], f32, tag="bsum1", name="bsum1")
    nc.vector.reduce_sum(out=bsum1[:], in_=bp1[:], axis=AX.X)
    bsums = (bsum0, bsum1)
    # uni bias column pre-scaled by 0.1/7
    bib_sc = []
    for ci, (m0, msz) in enumerate(CHUNKS):
        c2 = stp.tile([128, 1], f32, tag="stat", name="stat")
        nc.vector.tensor_scalar_mul(c2[0:msz, :], bsums[ci][0:msz, N:N + 1],
                                    0.1 / 7.0)
        bib_sc.append(c2)

    # ---- persistent buffers ----
    h1loc = cp.tile([HS, B], f32, tag="h1loc", name="h1loc")
    h2g = cp.tile([100, B], bf16, tag="h2g", name="h2g")
    A = [cp.tile([msz, B], bf16, tag=f"A{ci}", name=f"A{ci}")
         for ci, (m0, msz) in enumerate(CHUNKS)]
    z = [cp.tile([msz, B], bf16, tag=f"z{ci}", name=f"z{ci}")
         for ci, (m0, msz) in enumerate(CHUNKS)]
    outsb = [cp.tile([msz, B], bf16, tag=f"out{ci}", name=f"out{ci}")
             for ci, (m0, msz) in enumerate(CHUNKS)]
    h2b = cp.tile([100, B], bf16, tag="h2b", name="h2b")
    ex10 = cp.tile([11, B], f32r, tag="ex10", name="ex10")
    arv = cp.tile([1, 3 * B], f32, tag="arv", name="arv")
    tv = cp.tile([1, B], f32r, tag="tv", name="tv")
    uv = cp.tile([1, B], f32r, tag="uv", name="uv")
    xu = []
    for ci, (m0, msz) in enumerate(CHUNKS):
        xu.append(cp.tile([msz, B], bf16, tag=f"xu{ci}", name=f"xu{ci}"))

    cc1_in = drp.tile([1, B], f32, tag="cc1_in", name="cc1_in")
    # AR1 result laid out [128, 8] so the 1/||U|| reciprocal runs 128-wide
    # (a [1, B] row reciprocal is a 6.5us single-partition stall that Tile
    # schedules mid-pipeline on the DVE queue, starving the PE)
    cc1_out = drp.tile([128, B // 128], f32, tag="cc1_out",
                       addr_space="Shared", name="cc1_out")
    us_dram = drp.tile([1, B], f32r, tag="us_dram", name="us_dram")
    cc2_in = drp.tile([1, B], f32, tag="cc2_in", name="cc2_in")
    # AR2 result laid out [128, 8] so the tail rsqrt runs 128-wide
    cc2_out = drp.tile([128, B // 128], f32, tag="cc2_out",
                       addr_space="Shared", name="cc2_out")
    cc3_in = drp.tile([100, B], bf16, tag="cc3_in", name="cc3_in")
    cc3_out = drp.tile([100, B], bf16, tag="cc3_out",
                       addr_space="Shared", name="cc3_out")
    u_dram = drp.tile([1, B], f32r, tag="u_dram", name="u_dram")

    def stream_group(xsrc, wsrc):
        """DMA one group's x (one shot) + W (two halves), reduce W over e.

        The e-reduction is a pairwise add tree with expert slices contiguous
        along DS (step-1 bf16 operands hit the DVE 2x mode).
        """
        xt = xtp.tile([128, KT, B], bf16, tag="xt", name="xt")
        nc.sync.dma_start(xt[:], xsrc)
        ws = wsp.tile([128, KT, DS], bf16, tag="ws", name="ws")
        for h in range(2):
            wt = wtp.tile([128, KH, G, DS], bf16, tag="wt", name="wt")
            nc.sync.dma_start(wt[:], wsrc[:, h * KH:(h + 1) * KH, :, :])
            t01 = wrp.tile([128, KH, DS], bf16, tag="wred", name="wred")
            nc.vector.tensor_tensor(out=t01[:], in0=wt[:, :, 0, :],
                                    in1=wt[:, :, 1, :], op=ALU.add)
            t23 = wrp.tile([128, KH, DS], bf16, tag="wred", name="wred")
            nc.vector.tensor_tensor(out=t23[:], in0=wt[:, :, 2, :],
                                    in1=wt[:, :, 3, :], op=ALU.add)
            t45 = wrp.tile([128, KH, DS], bf16, tag="wred", name="wred")
            nc.vector.tensor_tensor(out=t45[:], in0=wt[:, :, 4, :],
                                    in1=wt[:, :, 5, :], op=ALU.add)
            nc.vector.tensor_tensor(out=t01[:], in0=t01[:], in1=t23[:],
                                    op=ALU.add)
            nc.vector.tensor_tensor(out=t45[:], in0=t45[:], in1=wt[:, :, 6, :],
                                    op=ALU.add)
            nc.vector.tensor_tensor(out=ws[:, h * KH:(h + 1) * KH, :],
                                    in0=t01[:], in1=t45[:], op=ALU.add)
        return xt, ws

    def group_matmuls(xt, ws, g):
        """raw = x @ Wsum (+ e-summed bias on the ACT copy); stage bf16."""
        raws = []
        for ci, (m0, msz) in enumerate(CHUNKS):
            gp = gs0 if ci == 0 else gs1
            raw = gp.tile([msz, B], bf16, tag=f"gs{ci}", name=f"gs{ci}")
            for bs in range(NBS):
                sl = slice(bs * BSL, (bs + 1) * BSL)
                ps = ps_tile(msz)
                for kt in range(KT):
                    nc.tensor.matmul(
                        ps[:],
                        lhsT=ws[:, kt, m0:m0 + msz],
                        rhs=xt[:, kt, sl],
                        start=(kt == 0), stop=(kt == KT - 1))
                nc.scalar.activation(raw[:, sl], ps[:], ACTF.Identity,
                                     bias=bsums[ci][0:msz, g:g + 1])
            raws.append(raw)
        return raws

    def h1_mms(ri, xt):
        """Local h1 partial for router group ri: [HS, B] over this core's
        64 neurons (contraction over this group's 1024 k)."""
        for bs in range(NBS):
            sl = slice(bs * BSL, (bs + 1) * BSL)
            ps = ps_tile(HS)
            for kt in range(KT):
                nc.tensor.matmul(
                    ps[:],
                    lhsT=rt[:, ri, kt, :],
                    rhs=xt[:, kt, sl],
                    start=(kt == 0), stop=(kt == KT - 1))
            if ri == 0:
                nc.scalar.copy(h1loc[:, sl], ps[:])
            else:
                nc.vector.tensor_tensor(out=h1loc[:, sl], in0=ps[:],
                                        in1=h1loc[:, sl], op=ALU.add)

    fold_state = {"first": True}

    def fold_group(raws, i):
        """A += e_i * raw. Gate row i is PE-broadcast to 128 partitions,
        ACT-staged to SBUF bf16 so the DVE folds run in 2x mode."""
        bcs = bcp.tile([128, B], bf16, tag="bc", name="bc")
        for bs in range(NBS):
            sl = slice(bs * BSL, (bs + 1) * BSL)
            bc = ps_tile(128)
            nc.tensor.matmul(bc[:], lhsT=sel[:, i, :],
                             rhs=ex10[:, sl], start=True, stop=True)
            nc.scalar.copy(bcs[:, sl], bc[:])
        first = fold_state["first"]
        fold_state["first"] = False
        for ci, (m0, msz) in enumerate(CHUNKS):
            if first:
                nc.vector.tensor_tensor(out=A[ci][:], in0=raws[ci][:],
                                        in1=bcs[0:msz, :], op=ALU.mult)
            else:
                tmp = ctb.tile([128, B], bf16, tag="ctb", name="ctb")
                nc.vector.tensor_tensor(out=tmp[0:msz, :], in0=raws[ci][:],
                                        in1=bcs[0:msz, :], op=ALU.mult)
                nc.vector.tensor_tensor(out=A[ci][:], in0=A[ci][:],
                                        in1=tmp[0:msz, :], op=ALU.add)

    def bn_act(tiles, out_tiles, nparts, func):
        """BatchNorm (training stats over free axis) + activation."""
        for t, to in zip(tiles, out_tiles):
            dump = b1k.tile([128, B], f32, tag="big1k", name="big1k")
            mnr = stp.tile([128, 1], f32, tag="stat", name="stat")
            nc.scalar.activation(dump[0:nparts, :], t[:], ACTF.Copy,
                                 accum_out=mnr[0:nparts, :])
            mn = stp.tile([128, 1], f32, tag="stat", name="stat")
            nc.scalar.mul(mn[0:nparts, :], mnr[0:nparts, :], 1.0 / B)
            sq = b1k.tile([128, B], f32, tag="big1k", name="big1k")
            ex2r = stp.tile([128, 1], f32, tag="stat", name="stat")
            nc.scalar.activation(sq[0:nparts, :], t[:], ACTF.Square,
                                 accum_out=ex2r[0:nparts, :])
            ex2 = stp.tile([128, 1], f32, tag="stat", name="stat")
            nc.scalar.mul(ex2[0:nparts, :], ex2r[0:nparts, :], 1.0 / B)
            var = stp.tile([128, 1], f32, tag="stat", name="stat")
            nc.vector.tensor_tensor(out=var[0:nparts, :], in0=mn[0:nparts, :],
                                    in1=mn[0:nparts, :], op=ALU.mult)
            nc.vector.tensor_tensor(out=var[0:nparts, :], in0=ex2[0:nparts, :],
                                    in1=var[0:nparts, :], op=ALU.subtract)
            nc.vector.tensor_scalar_add(var[0:nparts, :], var[0:nparts, :],
                                        EPS_BN)
            sd = stp.tile([128, 1], f32, tag="stat", name="stat")
            nc.scalar.sqrt(sd[0:nparts, :], var[0:nparts, :])
            rs = stp.tile([128, 1], f32, tag="stat", name="stat")
            nc.vector.reciprocal(rs[0:nparts, :], sd[0:nparts, :])
            nb = stp.tile([128, 1], f32, tag="stat", name="stat")
            nc.vector.tensor_tensor(out=nb[0:nparts, :], in0=mn[0:nparts, :],
                                    in1=rs[0:nparts, :], op=ALU.mult)
            nc.vector.tensor_scalar_mul(nb[0:nparts, :], nb[0:nparts, :], -1.0)
            nc.scalar.activation(to[:], t[:], func,
                                 bias=nb[0:nparts, :], scale=rs[0:nparts, :])

    # ================= router groups first (PE starts on h1 at ~8us) ======
    router_raws = {}
    for ri, g in enumerate(ROUTER_GS):
        xt, ws = stream_group(xT_d[g], w_d[g])
        h1_mms(ri, xt)
        router_raws[g] = group_matmuls(xt, ws, g)

    # bn+relu on local h1 slice, then this core's h2 partial through its
    # 64-row slice of rw2; the cross-core coupling is a small [100, B]
    # bf16 AllReduce instead of a 1MB h1 AllGather (bcp pool is not
    # contended until the folds start, long after this is consumed)
    h1b_loc = bcp.tile([128, B], bf16, tag="bc", name="bc")
    bn_act([h1loc], [h1b_loc[0:HS, :]], HS, ACTF.Relu)
    h2p = ctb.tile([128, B], bf16, tag="ctb", name="ctb")
    for bs in range(NBS):
        sl = slice(bs * BSL, (bs + 1) * BSL)
        ps = ps_tile(100)
        nc.tensor.matmul(ps[:], lhsT=rw2[:], rhs=h1b_loc[0:HS, sl],
                         start=True, stop=True)
        nc.scalar.copy(h2p[0:100, sl], ps[:])
    nc.sync.dma_start(cc3_in[:], h2p[0:100, :])
    nc.gpsimd.collective_compute(
        "AllReduce", ALU.add,
        ins=[cc3_in.opt()], outs=[cc3_out.opt()],
        replica_groups=[list(range(NCORES))])

    # ================= uni branch -> z, ssz partial, AR1 =================
    for ci, (m0, msz) in enumerate(CHUNKS):
        nc.sync.dma_start(xu[ci][:], xuT_d[m0:m0 + msz, :])
    xib_t, wib_s = stream_group(xibT_d, wib_d)
    for ci, (m0, msz) in enumerate(CHUNKS):
        for bs in range(NBS):
            sl = slice(bs * BSL, (bs + 1) * BSL)
            ps = ps_tile(msz)
            for kt in range(KT):
                nc.tensor.matmul(ps[:], lhsT=wib_s[:, kt, m0:m0 + msz],
                                 rhs=xib_t[:, kt, sl],
                                 start=(kt == 0), stop=(kt == KT - 1))
            # z = (raw + bias)*0.1/7 + 0.9*xu  (fused add on DVE)
            nc.scalar.activation(z[ci][:, sl], ps[:], ACTF.Identity,
                                 scale=0.1 / 7.0, bias=bib_sc[ci][0:msz, :])
            nc.vector.scalar_tensor_tensor(
                out=z[ci][:, sl], in0=xu[ci][:, sl], scalar=0.9,
                in1=z[ci][:, sl], op0=ALU.mult, op1=ALU.add)
    # ssz partial: sum_d z^2 -> arv slot 2 -> cc1_in
    for bs in range(NBS):
        sl = slice(bs * BSL, (bs + 1) * BSL)
        ps = ps_tile(1)
        for ci, (m0, msz) in enumerate(CHUNKS):
            sq = ctb.tile([128, B], bf16, tag="ctb", name="ctb")
            nc.scalar.square(sq[0:msz, 0:BSL], z[ci][:, sl])
            nc.tensor.matmul(ps[:], lhsT=onesb[0:msz, :],
                             rhs=sq[0:msz, 0:BSL],
                             start=(ci == 0), stop=(ci == len(CHUNKS) - 1))
        nc.scalar.copy(arv[:, B + bs * BSL:B + (bs + 1) * BSL], ps[:])
    nc.sync.dma_start(cc1_in[:], arv[:, B:2 * B])
    nc.gpsimd.collective_compute(
        "AllReduce", ALU.add,
        ins=[cc1_in.opt()], outs=[cc1_out.opt()],
        replica_groups=[list(range(NCORES))])

    # ===== stream groups 1,2,3,5 (PE cover for bn/AG/finalize latency) ====
    pre_raws = {}
    for g in (1, 2, 3):
        xt, ws = stream_group(xT_d[g], w_d[g])
        pre_raws[g] = group_matmuls(xt, ws, g)

    # ================= router finalize =================
    # h2 AllReduce result fetched on the gpsimd queue: its completion wait
    # must not head-of-line-block the sync (x/W streams) or scalar (ACT
    # staging) queues, and gpsimd has nothing urgent queued behind it.
    nc.gpsimd.dma_start(h2g[:], cc3_out[:])
    g5_xt, g5_ws = stream_group(xT_d[5], w_d[5])
    pre_raws[5] = group_matmuls(g5_xt, g5_ws, 5)
    bn_act([h2g], [h2b], 100, ACTF.Tanh)
    for bs in range(NBS):
        sl = slice(bs * BSL, (bs + 1) * BSL)
        ps = ps_tile(11)
        nc.tensor.matmul(ps[:], lhsT=rw3[:],
                         rhs=h2b[:, sl], start=True, stop=True)
        sg = ctp.tile([128, BSL], f32, tag="ctmp", name="ctmp")
        nc.scalar.activation(sg[0:11, :], ps[:], ACTF.Sigmoid, bias=rb3[:],
                             scale=1.0)
        nc.scalar.activation(ex10[:, sl], sg[0:11, :], ACTF.Exp, scale=10.0)

    # e10 row (no AR1 dependency): arv slot 3 <- 7*e10
    for bs in range(NBS):
        sl = slice(bs * BSL, (bs + 1) * BSL)
        ps = ps_tile(1)
        nc.tensor.matmul(ps[:], lhsT=sel[:, 10, 0:1],
                         rhs=ex10[:, sl], start=True, stop=True)
        nc.scalar.copy(arv[:, 2 * B + bs * BSL:2 * B + (bs + 1) * BSL], ps[:])
    e10v = arv[:, 2 * B:3 * B]
    nc.vector.tensor_scalar_mul(e10v, e10v, 7.0)

    # ================= folds: staged groups, then stream the rest =========
    for g in (0, 4, 8, 1, 2, 3):
        fold_group(router_raws.get(g) or pre_raws[g], g)
    fold_group(pre_raws[5], 5)

    for g in (6, 7, 9):
        xt, ws = stream_group(xT_d[g], w_d[g])
        raws = group_matmuls(xt, ws, g)
        fold_group(raws, g)

    # AR1 result -> 1/||U|| on [128, 8] (cheap 128-wide recip), bounced
    # through DRAM back into [1, B] row form for the gate-row arithmetic.
    # All on the gpsimd queue, which has nothing urgent behind this.
    s8 = cp.tile([128, B // 128], f32, tag="s8", name="s8")
    nc.gpsimd.dma_start(s8[:], cc1_out[:])
    nc.scalar.sqrt(s8[:], s8[:])
    nc.vector.tensor_scalar_max(s8[:], s8[:], EPS_NORM)
    s8r = cp.tile([128, B // 128], f32r, tag="s8r", name="s8r")
    nc.vector.reciprocal(s8r[:], s8[:])
    nc.gpsimd.dma_start(us_dram[:], s8r[:])
    nc.gpsimd.dma_start(uv[:], us_dram[:])
    nc.vector.tensor_tensor(out=tv[:], in0=e10v, in1=uv[:], op=ALU.mult)
    # e10sq/8: each core contributes 1/8 so the AR2 sum restores it
    nc.vector.tensor_tensor(out=e10v, in0=e10v, in1=e10v, op=ALU.mult)
    nc.vector.tensor_scalar_mul(e10v, e10v, 0.125)

    # ========= tail: q_loc = |A|^2 + t*(2 A.z) + (7e10)^2/8 -> AR2 ========
    for bs in range(NBS):
        sl = slice(bs * BSL, (bs + 1) * BSL)
        psa = ps_tile(1)
        psc = ps_tile(1)
        for ci, (m0, msz) in enumerate(CHUNKS):
            sqa = ctb.tile([128, B], bf16, tag="ctb", name="ctb")
            nc.scalar.square(sqa[0:msz, 0:BSL], A[ci][:, sl])
            nc.tensor.matmul(psa[:], lhsT=onesb[0:msz, :],
                             rhs=sqa[0:msz, 0:BSL],
                             start=(ci == 0), stop=(ci == len(CHUNKS) - 1))
            cza = ctb.tile([128, B], bf16, tag="ctb", name="ctb")
            nc.vector.tensor_tensor(out=cza[0:msz, 0:BSL], in0=A[ci][:, sl],
                                    in1=z[ci][:, sl], op=ALU.mult)
            nc.tensor.matmul(psc[:], lhsT=twosb[0:msz, :],
                             rhs=cza[0:msz, 0:BSL],
                             start=(ci == 0), stop=(ci == len(CHUNKS) - 1))
        qt = ctp.tile([128, BSL], f32, tag="ctmp", name="ctmp")
        nc.vector.tensor_tensor(out=qt[0:1, :], in0=psc[:], in1=tv[:, sl],
                                op=ALU.mult)
        nc.vector.tensor_tensor(out=qt[0:1, :], in0=qt[0:1, :], in1=psa[:],
                                op=ALU.add)
        nc.vector.tensor_tensor(out=arv[:, bs * BSL:(bs + 1) * BSL],
                                in0=qt[0:1, :],
                                in1=arv[:, 2 * B + bs * BSL:2 * B + (bs + 1) * BSL],
                                op=ALU.add)
    nc.sync.dma_start(cc2_in[:], arv[:, 0:B])
    nc.gpsimd.collective_compute(
        "AllReduce", ALU.add,
        ins=[cc2_in.opt()], outs=[cc2_out.opt()],
        replica_groups=[list(range(NCORES))])

    # P = A + t*z, computed while AR2 is in flight
    tvb = bcp.tile([128, B], bf16, tag="bc", name="bc")
    for bs in range(NBS):
        sl = slice(bs * BSL, (bs + 1) * BSL)
        btv = ps_tile(128)
        nc.tensor.matmul(btv[:], lhsT=ones1[:],
                         rhs=tv[:, sl], start=True, stop=True)
        nc.scalar.copy(tvb[:, sl], btv[:])
    for ci, (m0, msz) in enumerate(CHUNKS):
        tmp = ctb.tile([128, B], bf16, tag="ctb", name="ctb")
        nc.vector.tensor_tensor(out=tmp[0:msz, :], in0=z[ci][:],
                                in1=tvb[0:msz, :], op=ALU.mult)
        nc.vector.tensor_tensor(out=A[ci][:], in0=A[ci][:],
                                in1=tmp[0:msz, :], op=ALU.add)

    # sqrt ACT table preload during AR2 flight: the tail sqrt otherwise
    # pays a ~1.5us ACT_TABLE_LOAD on the critical path
    dwarm = stp.tile([128, 1], f32, tag="stat", name="stat")
    nc.scalar.sqrt(dwarm[0:1, :], bib_sc[0][0:1, :])

    # q -> u = 1/max(sqrt(q), eps) on [128, 8] (a [1, B] row reciprocal is
    # a 7.8us single-partition iterative divide; 128-wide it is ~0.1us),
    # then a DRAM bounce turns u back into a [1, B] row for the broadcast
    u8 = cp.tile([128, B // 128], f32, tag="u8", name="u8")
    nc.sync.dma_start(u8[:], cc2_out[:])
    nc.scalar.sqrt(u8[:], u8[:])
    nc.vector.tensor_scalar_max(u8[:], u8[:], EPS_NORM)
    u8r = cp.tile([128, B // 128], f32r, tag="u8r", name="u8r")
    nc.vector.reciprocal(u8r[:], u8[:])
    nc.sync.dma_start(u_dram[:], u8r[:])
    ubrow = cp.tile([1, B], f32r, tag="ubrow", name="ubrow")
    nc.sync.dma_start(ubrow[:], u_dram[:])
    ub = bcp.tile([128, B], bf16, tag="bc", name="bc")
    for bs in range(NBS):
        sl = slice(bs * BSL, (bs + 1) * BSL)
        bu = ps_tile(128)
        nc.tensor.matmul(bu[:], lhsT=ones1[:],
                         rhs=ubrow[:, sl], start=True, stop=True)
        nc.scalar.copy(ub[:, sl], bu[:])
    for ci, (m0, msz) in enumerate(CHUNKS):
        nc.vector.tensor_tensor(out=outsb[ci][:], in0=A[ci][:],
                                in1=ub[0:msz, :], op=ALU.mult)
        nc.sync.dma_start(outT_d[m0:m0 + msz, :], outsb[ci][:])

    for p in reversed(list(pools.values())):
        p.__exit__(None, None, None)


def _build_nc():
    nc = bacc.Bacc("TRN2", target_bir_lowering=False, debug=False,
                   num_devices=NCORES)
    with tile.TileContext(nc) as tc:
        with nc.allow_low_precision(reason="bf16 streams / f32r reductions are intentional"):
            _emit(nc, tc)
    nc.compile()
    return nc


def _as_bf16(a):
    return np.ascontiguousarray(a.astype(ml_dtypes.bfloat16))


def _host_prep(inputs):
    x_enc = np.asarray(inputs["x_enc"], dtype=np.float32)
    x_ib = np.asarray(inputs["x_ib"], dtype=np.float32)
    x_uni = np.asarray(inputs["x_uni"], dtype=np.float32)
    W_proj = np.asarray(inputs["W_proj"], dtype=np.float32)
    b_proj = np.asarray(inputs["b_proj"], dtype=np.float32)
    W_ib = np.asarray(inputs["W_ib"], dtype=np.float32)
    b_ib = np.asarray(inputs["b_ib"], dtype=np.float32)

    # x_enc [N,B,K] -> [N, 128, KT, B] partition-major bf16
    xT = _as_bf16(x_enc.transpose(0, 2, 1).reshape(N, KT, 128, B).transpose(0, 2, 1, 3))
    # x_ib [B,K] -> [128, KT, B]
    xibT = _as_bf16(x_ib.T.reshape(KT, 128, B).transpose(1, 0, 2))
    sel = np.zeros((11, 11, 128), dtype=np.float32)
    for q in range(11):
        sel[q, q, :] = 1.0
    rb3 = np.ascontiguousarray(np.asarray(inputs["r_b3"], np.float32).reshape(11, 1))
    # r_w1 [3072, 512]: per-core h-slice -> [128, 3, KT, 64]
    rw1_full = np.asarray(inputs["r_w1"], np.float32)
    rw2_full = np.asarray(inputs["r_w2"], np.float32)
    rw3 = _as_bf16(np.asarray(inputs["r_w3"], np.float32))
    ones_host = np.ones((1, 128), dtype=np.float32)
    onesb16 = np.ones((128, 2), dtype=np.float32)
    onesb16[:, 1] = 2.0
    onesb16 = _as_bf16(onesb16)

    in_maps = []
    for c in range(NCORES):
        ds = slice(c * DS, (c + 1) * DS)
        # W_proj [N,G,K,D] ds-slice -> [N, 128, KT, G, DS]
        wc = _as_bf16(W_proj[:, :, :, ds].reshape(N, G, KT, 128, DS)
                      .transpose(0, 3, 2, 1, 4))
        wibc = _as_bf16(W_ib[:, :, ds].reshape(G, KT, 128, DS)
                        .transpose(2, 1, 0, 3))
        # biases: [DS, 11, G] with group 10 = b_ib, split into chunks
        bp_full = np.zeros((DS, N + 1, G), dtype=np.float32)
        bp_full[:, :N, :] = b_proj[:, :, ds].transpose(2, 0, 1)
        bp_full[:, N, :] = b_ib[:, ds].T
        bp_full = _as_bf16(bp_full)
        rw1c = _as_bf16(rw1_full[:, c * HS:(c + 1) * HS]
                        .reshape(3, KT, 128, HS).transpose(2, 0, 1, 3))
        rw2c = _as_bf16(rw2_full[c * HS:(c + 1) * HS, :])
        in_maps.append({
            "xT": xT,
            "xibT": xibT,
            "w": wc,
            "wib": wibc,
            "xuT": _as_bf16(x_uni[:, ds].T),
            "bp0": bp_full[0:128],
            "bp1": bp_full[128:192],
            "rw1": rw1c,
            "rw2": rw2c,
            "rw3": rw3,
            "rb3": rb3,
            "sel": sel,
            "onesd": ones_host,
            "onesb16": onesb16,
        })
    return in_maps


def kernel(**inputs):
    global LAST_RESULTS
    if "nc" not in _NC_CACHE:
        _NC_CACHE["nc"] = _build_nc()
    nc = _NC_CACHE["nc"]
    in_maps = _host_prep(inputs)
    res = run_bass_kernel_spmd(nc, in_maps, list(range(NCORES)))
    LAST_RESULTS = res
    full = np.concatenate(
        [res.results[c]["outT"].astype(np.float32) for c in range(NCORES)],
        axis=0)
    return np.ascontiguousarray(full.T)
